# revision 6
# baseline (speedup 1.0000x reference)
"""BERT self-attention on 8 Trainium2 NeuronCores.

Sharding: data-parallel over batch (B=8 -> one batch element per core).
Each core computes full self-attention for its batch element:
  Q/K/V projections, per-head softmax(Q K^T / 8 + mask) V, output proj.

Layout strategy (per core):
  - Host passes xT = x.T [768,1024] and W.T [768,768] so every matmul
    contracts over the partition axis.
  - QT,KT [d, L] and V [L, d] are produced directly by the projections.
  - Attention runs transposed: ST[k,q] = K Q^T per head, so softmax's
    reduction axis (k) lands on partitions: exp via ScalarE with the
    attention mask as per-partition bias (no max subtraction: scores are
    ~N(0,1), |s|<~7, exp is safe in fp32); the denominator comes from a
    ones column appended to V (out row 64); P^T V accumulates ctx^T
    [d, q] which feeds the output projection as lhsT directly.
  - Matmul inputs are float32r (full PE speed at N>=512), accumulation
    and softmax in fp32.
"""

import numpy as np

import concourse.bass as bass  # noqa: F401
import concourse.mybir as mybir
import concourse.tile as tile
from concourse import bacc
from concourse.bass_interp import get_hw_module
from concourse.bass_utils import run_bass_kernel_spmd

B, L, H = 8, 1024, 768
NH, HD = 12, 64
NC = H // 128          # 6 chunks of hidden dim
LC = L // 128          # 8 chunks of sequence dim
F32 = mybir.dt.float32
EXP = mybir.ActivationFunctionType.Exp


def build_bass(compute_rounded: bool = True):
    CDT = mybir.dt.float32r if compute_rounded else F32

    nc = bacc.Bacc("TRN2", debug=False, num_devices=8)

    xt_e = nc.declare_dram_parameter("xt", [H, L], CDT, isOutput=False)
    wqt_e = nc.declare_dram_parameter("wqt", [H, H], CDT, isOutput=False)
    wkt_e = nc.declare_dram_parameter("wkt", [H, H], CDT, isOutput=False)
    wvt_e = nc.declare_dram_parameter("wvt", [H, H], CDT, isOutput=False)
    wot_e = nc.declare_dram_parameter("wot", [H, H], CDT, isOutput=False)
    bq_e = nc.declare_dram_parameter("bq", [H], F32, isOutput=False)
    bk_e = nc.declare_dram_parameter("bk", [H], F32, isOutput=False)
    bv_e = nc.declare_dram_parameter("bv", [H], CDT, isOutput=False)
    bo_e = nc.declare_dram_parameter("bo", [H], CDT, isOutput=False)
    mask_e = nc.declare_dram_parameter("mask", [L], F32, isOutput=False)
    out_e = nc.declare_dram_parameter("out", [L, H], F32, isOutput=True)

    with tile.TileContext(nc) as tc:
        with (
            tc.tile_pool(name="small", bufs=1) as small,
            tc.tile_pool(name="acts", bufs=1) as acts,
            tc.tile_pool(name="outp", bufs=2) as out_pool,
            tc.tile_pool(name="psA", bufs=2, space="PSUM") as psA,
            tc.tile_pool(name="psB", bufs=2, space="PSUM") as psB,
        ):
            # ---- constants / small tensors ----
            mask_sb = small.tile([128, LC], F32)
            nc.sync.dma_start(mask_sb[:], mask_e[:].rearrange("(c p) -> p c", p=128))
            bq_sb = small.tile([128, NC], F32)
            nc.sync.dma_start(bq_sb[:], bq_e[:].rearrange("(c p) -> p c", p=128))
            bk_sb = small.tile([128, NC], F32)
            nc.sync.dma_start(bk_sb[:], bk_e[:].rearrange("(c p) -> p c", p=128))
            bv_sb = small.tile([1, H], CDT)
            nc.sync.dma_start(bv_sb[:], bv_e[None, :])
            bo_sb = small.tile([1, H], CDT)
            nc.sync.dma_start(bo_sb[:], bo_e[None, :])
            ones32 = small.tile([128, 128], F32)
            nc.vector.memset(ones32[:], 1.0)
            ones = small.tile([128, 128], CDT)
            nc.vector.tensor_copy(ones[:], ones32[:])

            qt_sb = acts.tile([128, NC, L], CDT)
            kt_sb = acts.tile([128, NC, L], CDT)
            v_sb = acts.tile([128, LC, NH, HD + 1], CDT)  # [..., 64] = ones col
            ctxt_sb = acts.tile([128, NC, L], CDT)

            nc.vector.tensor_copy(
                v_sb[:, :, :, HD],
                ones32[:, 0 : LC * NH].rearrange("p (a b) -> p a b", a=LC),
            )

            # =========== projection phase (xt + wv/wk/wq scoped) ===========
            with (
                tc.tile_pool(name="xt", bufs=1) as xt_pool,
                tc.tile_pool(name="w1", bufs=2) as w1,
            ):
                xt_sb = xt_pool.tile([128, NC, L], CDT)
                for c in range(NC):
                    nc.sync.dma_start(
                        xt_sb[:, c, :],
                        xt_e[:].rearrange("(c p) q -> p c q", p=128)[:, c, :],
                    )

                # ---- V projection: V[l, d] = x Wv^T + bv (natural layout)
                wv_sb = w1.tile([128, NC, H], CDT, tag="w")
                for c in range(NC):
                    nc.sync.dma_start(
                        wv_sb[:, c, :],
                        wvt_e[:].rearrange("(c p) d -> p c d", p=128)[:, c, :],
                    )
                for lc in range(LC):
                    ps = psA.tile([128, 1024], F32, tag="psA")
                    for off, width in ((0, 512), (512, 256)):
                        for kc in range(NC):
                            nc.tensor.matmul(
                                ps[:, off : off + width],
                                xt_sb[:, kc, lc * 128 : lc * 128 + 128],
                                wv_sb[:, kc, off : off + width],
                                start=(kc == 0),
                                stop=False,
                            )
                        nc.tensor.matmul(  # + bv (ones row x bias row)
                            ps[:, off : off + width],
                            ones[0:1, 0:128],
                            bv_sb[0:1, off : off + width],
                            start=False,
                            stop=True,
                        )
                    nc.vector.tensor_copy(
                        v_sb[:, lc, :, 0:HD],
                        ps[:, 0:H].rearrange("p (h d) -> p h d", d=HD),
                    )

                # ---- K^T then Q^T projections: out[d, q] = W x^T + b
                for w_e, b_sb, dst in ((wkt_e, bk_sb, kt_sb), (wqt_e, bq_sb, qt_sb)):
                    w_sb = w1.tile([128, NC, H], CDT, tag="w")
                    for c in range(NC):
                        nc.sync.dma_start(
                            w_sb[:, c, :],
                            w_e[:].rearrange("(c p) d -> p c d", p=128)[:, c, :],
                        )
                    for dc in range(NC):
                        ps = psA.tile([128, 1024], F32, tag="psA")
                        for qh in range(2):
                            o = qh * 512
                            for kc in range(NC):
                                nc.tensor.matmul(
                                    ps[:, o : o + 512],
                                    w_sb[:, kc, dc * 128 : dc * 128 + 128],
                                    xt_sb[:, kc, o : o + 512],
                                    start=(kc == 0),
                                    stop=(kc == NC - 1),
                                )
                        nc.vector.tensor_scalar_add(
                            dst[:, dc, :], ps[:, :], b_sb[:, dc : dc + 1]
                        )

            # =========== attention + output projection ===========
            with (
                tc.tile_pool(name="w2", bufs=1) as w2,
                tc.tile_pool(name="et", bufs=3) as et_pool,
                tc.tile_pool(name="norm", bufs=2) as norm_pool,
            ):
                wo_sb = w2.tile([128, NC, H], CDT)
                for c in range(NC):
                    nc.sync.dma_start(
                        wo_sb[:, c, :],
                        wot_e[:].rearrange("(c p) d -> p c d", p=128)[:, c, :],
                    )

                for hp in range(NH // 2):
                    ha, hb = 2 * hp, 2 * hp + 1
                    # per head: rows 0:64 = ctx^T, row 64 = softmax denominator
                    ctx_a = psB.tile([128, 1024], F32, tag="psB")
                    ctx_b = psB.tile([128, 1024], F32, tag="psB")
                    for kc in range(LC):
                        st_a = psA.tile([128, 1024], F32, tag="psA")
                        st_b = psA.tile([128, 1024], F32, tag="psA")
                        for qh in range(2):
                            o = qh * 512
                            # S^T[k, q] = K Q^T for both heads (row-group packed)
                            nc.tensor.matmul(
                                st_a[:, o : o + 512],
                                kt_sb[0:64, hp, kc * 128 : kc * 128 + 128],
                                qt_sb[0:64, hp, o : o + 512],
                                start=True,
                                stop=True,
                            )
                            nc.tensor.matmul(
                                st_b[:, o : o + 512],
                                kt_sb[64:128, hp, kc * 128 : kc * 128 + 128],
                                qt_sb[64:128, hp, o : o + 512],
                                start=True,
                                stop=True,
                            )
                        # P^T = exp(S^T/8 + mask_k)
                        et_a = et_pool.tile([128, 1024], CDT, tag="et")
                        et_b = et_pool.tile([128, 1024], CDT, tag="et")
                        nc.scalar.activation(
                            et_a[:], st_a[:], EXP,
                            bias=mask_sb[:, kc : kc + 1], scale=0.125,
                        )
                        nc.scalar.activation(
                            et_b[:], st_b[:], EXP,
                            bias=mask_sb[:, kc : kc + 1], scale=0.125,
                        )
                        first, last = kc == 0, kc == LC - 1
                        for qh in range(2):
                            o = qh * 512
                            # ctx^T[d, q] += V^T P^T ; row 64 = denominator
                            nc.tensor.matmul(
                                ctx_a[0 : HD + 1, o : o + 512],
                                v_sb[:, kc, ha, :],
                                et_a[:, o : o + 512],
                                start=first,
                                stop=last,
                            )
                            nc.tensor.matmul(
                                ctx_b[0 : HD + 1, o : o + 512],
                                v_sb[:, kc, hb, :],
                                et_b[:, o : o + 512],
                                start=first,
                                stop=last,
                            )
                    # normalize: ctx^T[d, q] *= 1/denom[q] (broadcast via matmul)
                    recip = norm_pool.tile([65, 1024], CDT, tag="recip")
                    with nc.allow_low_precision(reason="feeds f32r matmul"):
                        nc.vector.reciprocal(recip[64:65, :], ctx_a[64:65, :])
                    bc_a = psA.tile([64, 1024], F32, tag="psA")
                    for o in (0, 512):
                        nc.tensor.matmul(
                            bc_a[:, o : o + 512],
                            ones[64:65, 0:64],
                            recip[64:65, o : o + 512],
                            start=True,
                            stop=True,
                        )
                    bc_a_sb = norm_pool.tile([64, 1024], F32, tag="bc")
                    nc.vector.tensor_copy(bc_a_sb[:], bc_a[:])
                    nc.vector.tensor_tensor(
                        ctxt_sb[0:64, hp, :], ctx_a[0:64, :], bc_a_sb[:],
                        mybir.AluOpType.mult,
                    )

                    recip_b = norm_pool.tile([65, 1024], CDT, tag="recip")
                    with nc.allow_low_precision(reason="feeds f32r matmul"):
                        nc.vector.reciprocal(recip_b[64:65, :], ctx_b[64:65, :])
                    bc_b = psA.tile([64, 1024], F32, tag="psA")
                    for o in (0, 512):
                        nc.tensor.matmul(
                            bc_b[:, o : o + 512],
                            ones[64:65, 0:64],
                            recip_b[64:65, o : o + 512],
                            start=True,
                            stop=True,
                        )
                    bc_b_sb = norm_pool.tile([64, 1024], F32, tag="bc")
                    nc.vector.tensor_copy(bc_b_sb[:], bc_b[:])
                    tmp_o = norm_pool.tile([64, 1024], CDT, tag="tmp")
                    nc.vector.tensor_tensor(
                        tmp_o[:], ctx_b[0:64, :], bc_b_sb[:], mybir.AluOpType.mult
                    )
                    # lift odd head to partitions 64:128 (DMA moves across
                    # partitions; DVE cannot)
                    nc.sync.dma_start(ctxt_sb[64:128, hp, :], tmp_o[:])

                # ---- output projection: out[q, o] = ctx Wo^T + bo ----
                for lc in range(LC):
                    ps = psA.tile([128, 1024], F32, tag="psA")
                    for off, width in ((0, 512), (512, 256)):
                        for c in range(NC):
                            nc.tensor.matmul(
                                ps[:, off : off + width],
                                ctxt_sb[:, c, lc * 128 : lc * 128 + 128],
                                wo_sb[:, c, off : off + width],
                                start=(c == 0),
                                stop=False,
                            )
                        nc.tensor.matmul(  # + bo
                            ps[:, off : off + width],
                            ones[0:1, 0:128],
                            bo_sb[0:1, off : off + width],
                            start=False,
                            stop=True,
                        )
                    o_sb = out_pool.tile([128, H], F32, tag="outp")
                    nc.vector.tensor_copy(o_sb[:], ps[:, 0:H])
                    nc.sync.dma_start(out_e[lc * 128 : lc * 128 + 128, :], o_sb[:])

    nc.finalize()
    nc.m = get_hw_module(nc.m)
    return nc


_NC_CACHE = {}


def _get_nc(compute_rounded: bool = True):
    if compute_rounded not in _NC_CACHE:
        _NC_CACHE[compute_rounded] = build_bass(compute_rounded)
    return _NC_CACHE[compute_rounded]


def make_in_maps(inputs):
    f = lambda a: np.ascontiguousarray(np.asarray(a, dtype=np.float32))  # noqa: E731
    hs = f(inputs["hidden_states"])
    mask = f(inputs["attention_mask"]).reshape(B, L)
    shared = {
        "wqt": f(np.asarray(inputs["Wq"]).T),
        "wkt": f(np.asarray(inputs["Wk"]).T),
        "wvt": f(np.asarray(inputs["Wv"]).T),
        "wot": f(np.asarray(inputs["Wo"]).T),
        "bq": f(inputs["bq"]),
        "bk": f(inputs["bk"]),
        "bv": f(inputs["bv"]),
        "bo": f(inputs["bo"]),
    }
    return [
        {"xt": f(hs[b].T), "mask": mask[b], **shared}
        for b in range(B)
    ]


def run_spmd(inputs, trace=False, compute_rounded=True):
    nc = _get_nc(compute_rounded)
    res = run_bass_kernel_spmd(nc, make_in_maps(inputs), list(range(B)), trace=trace)
    out = np.stack([res.results[b]["out"] for b in range(B)]).astype(np.float32)
    return out, res


def kernel(**inputs) -> np.ndarray:
    out, _ = run_spmd(inputs, trace=False)
    return out


# revision 9
# speedup vs baseline: 1.0624x; 1.0624x over previous
"""BERT self-attention on 8 Trainium2 NeuronCores.

Sharding: data-parallel over batch (B=8 -> one batch element per core).
Each core computes full self-attention for its batch element:
  Q/K/V projections, per-head softmax(Q K^T / 8 + mask) V, output proj.

Layout strategy (per core):
  - Host passes xT = x.T [768,1024] and W.T [768,768] so every matmul
    contracts over the partition axis.
  - QT,KT [d, L] and V [L, d] are produced directly by the projections.
  - Attention runs transposed: ST[k,q] = K Q^T per head, so softmax's
    reduction axis (k) lands on partitions: exp via ScalarE with the
    attention mask as per-partition bias (no max subtraction: scores are
    ~N(0,1), |s|<~7, exp is safe in fp32); the denominator comes from a
    ones column appended to V (out row 64); P^T V accumulates ctx^T
    [d, q] which feeds the output projection as lhsT directly.
  - Matmul inputs are float32r (full PE speed at N>=512), accumulation
    and softmax in fp32.
"""

import numpy as np

import concourse.bass as bass  # noqa: F401
import concourse.mybir as mybir
import concourse.tile as tile
from concourse import bacc
from concourse.bass_interp import get_hw_module
from concourse.bass_utils import run_bass_kernel_spmd

B, L, H = 8, 1024, 768
NH, HD = 12, 64
NC = H // 128          # 6 chunks of hidden dim
LC = L // 128          # 8 chunks of sequence dim
F32 = mybir.dt.float32
EXP = mybir.ActivationFunctionType.Exp


def build_bass(compute_rounded: bool = True):
    CDT = mybir.dt.float32r if compute_rounded else F32

    nc = bacc.Bacc("TRN2", debug=False, num_devices=8)

    xt_e = nc.declare_dram_parameter("xt", [H, L], CDT, isOutput=False)
    wqt_e = nc.declare_dram_parameter("wqt", [H, H], CDT, isOutput=False)
    wkt_e = nc.declare_dram_parameter("wkt", [H, H], CDT, isOutput=False)
    wvt_e = nc.declare_dram_parameter("wvt", [H, H], CDT, isOutput=False)
    wot_e = nc.declare_dram_parameter("wot", [H, H], CDT, isOutput=False)
    bq_e = nc.declare_dram_parameter("bq", [H], F32, isOutput=False)
    bk_e = nc.declare_dram_parameter("bk", [H], F32, isOutput=False)
    bv_e = nc.declare_dram_parameter("bv", [H], CDT, isOutput=False)
    bo_e = nc.declare_dram_parameter("bo", [H], CDT, isOutput=False)
    mask_e = nc.declare_dram_parameter("mask", [L], F32, isOutput=False)
    out_e = nc.declare_dram_parameter("out", [L, H], F32, isOutput=True)

    with tile.TileContext(nc) as tc:
        with (
            tc.tile_pool(name="small", bufs=1) as small,
            tc.tile_pool(name="acts", bufs=1) as acts,
            tc.tile_pool(name="outp", bufs=2) as out_pool,
            tc.tile_pool(name="psA", bufs=2, space="PSUM") as psA,
            tc.tile_pool(name="psB", bufs=2, space="PSUM") as psB,
        ):
            # ---- constants / small tensors ----
            mask_sb = small.tile([128, LC], F32)
            nc.sync.dma_start(mask_sb[:], mask_e[:].rearrange("(c p) -> p c", p=128))
            bq_sb = small.tile([128, NC], F32)
            nc.sync.dma_start(bq_sb[:], bq_e[:].rearrange("(c p) -> p c", p=128))
            bk_sb = small.tile([128, NC], F32)
            nc.sync.dma_start(bk_sb[:], bk_e[:].rearrange("(c p) -> p c", p=128))
            bv_sb = small.tile([1, H], CDT)
            nc.sync.dma_start(bv_sb[:], bv_e[None, :])
            bo_sb = small.tile([1, H], CDT)
            nc.sync.dma_start(bo_sb[:], bo_e[None, :])
            ones32 = small.tile([128, 128], F32)
            nc.vector.memset(ones32[:], 1.0)
            ones = small.tile([128, 128], CDT)
            nc.vector.tensor_copy(ones[:], ones32[:])

            qt_sb = acts.tile([128, NC, L], CDT)
            kt_sb = acts.tile([128, NC, L], CDT)
            v_sb = acts.tile([128, LC, NH, HD + 1], CDT)  # [..., 64] = ones col
            ctxt_sb = acts.tile([128, NC, L], CDT)

            nc.vector.tensor_copy(
                v_sb[:, :, :, HD],
                ones32[:, 0 : LC * NH].rearrange("p (a b) -> p a b", a=LC),
            )

            # =========== projection phase (xt + wv/wk/wq scoped) ===========
            with (
                tc.tile_pool(name="xt", bufs=1) as xt_pool,
                tc.tile_pool(name="w1", bufs=2) as w1,
            ):
                xt_sb = xt_pool.tile([128, NC, L], CDT)
                for c in range(NC):
                    nc.sync.dma_start(
                        xt_sb[:, c, :],
                        xt_e[:].rearrange("(c p) q -> p c q", p=128)[:, c, :],
                    )

                # ---- V projection: V[l, d] = x Wv^T + bv (natural layout)
                wv_sb = w1.tile([128, NC, H], CDT, tag="w")
                for c in range(NC):
                    nc.sync.dma_start(
                        wv_sb[:, c, :],
                        wvt_e[:].rearrange("(c p) d -> p c d", p=128)[:, c, :],
                    )
                for lc in range(LC):
                    ps = psA.tile([128, 1024], F32, tag="psA")
                    for off, width in ((0, 512), (512, 256)):
                        for kc in range(NC):
                            nc.tensor.matmul(
                                ps[:, off : off + width],
                                xt_sb[:, kc, lc * 128 : lc * 128 + 128],
                                wv_sb[:, kc, off : off + width],
                                start=(kc == 0),
                                stop=False,
                            )
                        nc.tensor.matmul(  # + bv (ones row x bias row)
                            ps[:, off : off + width],
                            ones[0:1, 0:128],
                            bv_sb[0:1, off : off + width],
                            start=False,
                            stop=True,
                        )
                    nc.vector.tensor_copy(
                        v_sb[:, lc, :, 0:HD],
                        ps[:, 0:H].rearrange("p (h d) -> p h d", d=HD),
                    )

                # ---- K^T then Q^T projections: out[d, q] = W x^T + b
                for w_e, b_sb, dst in ((wkt_e, bk_sb, kt_sb), (wqt_e, bq_sb, qt_sb)):
                    w_sb = w1.tile([128, NC, H], CDT, tag="w")
                    for c in range(NC):
                        nc.sync.dma_start(
                            w_sb[:, c, :],
                            w_e[:].rearrange("(c p) d -> p c d", p=128)[:, c, :],
                        )
                    for dc in range(NC):
                        ps = psA.tile([128, 1024], F32, tag="psA")
                        for qh in range(2):
                            o = qh * 512
                            for kc in range(NC):
                                nc.tensor.matmul(
                                    ps[:, o : o + 512],
                                    w_sb[:, kc, dc * 128 : dc * 128 + 128],
                                    xt_sb[:, kc, o : o + 512],
                                    start=(kc == 0),
                                    stop=(kc == NC - 1),
                                )
                        nc.vector.tensor_scalar_add(
                            dst[:, dc, :], ps[:, :], b_sb[:, dc : dc + 1]
                        )

            # =========== attention + output projection ===========
            with (
                tc.tile_pool(name="w2", bufs=1) as w2,
                tc.tile_pool(name="et", bufs=6) as et_pool,
                tc.tile_pool(name="norm", bufs=1) as norm_pool,
            ):
                wo_sb = w2.tile([128, NC, H], CDT)
                for c in range(NC):
                    nc.sync.dma_start(
                        wo_sb[:, c, :],
                        wot_e[:].rearrange("(c p) d -> p c d", p=128)[:, c, :],
                    )

                def make_normalize(hp, ctxu_a, ctxu_b, ra, rb):
                    def emit():
                        # broadcast 1/denom over 64 partitions via fp32 matmul,
                        # then scale ctx^T and store to ctxt_sb
                        for recip, ctxu, btag in ((ra, ctxu_a, "bca"),
                                                  (rb, ctxu_b, "bcb")):
                            bc = psA.tile([64, 1024], F32, tag="psA")
                            for o in (0, 512):
                                nc.tensor.matmul(
                                    bc[:, o : o + 512],
                                    ones32[64:65, 0:64],
                                    recip[64:65, o : o + 512],
                                    start=True,
                                    stop=True,
                                )
                            bc_sb = norm_pool.tile([64, 1024], F32, tag=btag)
                            nc.vector.tensor_copy(bc_sb[:], bc[:])
                            if btag == "bca":
                                nc.vector.tensor_tensor(
                                    ctxt_sb[0:64, hp, :], ctxu[0:64, :], bc_sb[:],
                                    mybir.AluOpType.mult,
                                )
                            else:
                                tmp_o = norm_pool.tile([64, 1024], CDT, tag="tmp")
                                nc.vector.tensor_tensor(
                                    tmp_o[:], ctxu[0:64, :], bc_sb[:],
                                    mybir.AluOpType.mult,
                                )
                                # lift odd head to partitions 64:128 (DMA can
                                # cross partitions; DVE cannot)
                                nc.sync.dma_start(ctxt_sb[64:128, hp, :], tmp_o[:])
                    return emit

                pending = None
                for hp in range(NH // 2):
                    ha, hb = 2 * hp, 2 * hp + 1
                    # per head: rows 0:64 = ctx^T, row 64 = softmax denominator
                    ctx_a = psB.tile([128, 1024], F32, tag="psB")
                    ctx_b = psB.tile([128, 1024], F32, tag="psB")

                    def emit_pv(kc, et_a, et_b, ctx_a=ctx_a, ctx_b=ctx_b,
                                ha=ha, hb=hb):
                        first, last = kc == 0, kc == LC - 1
                        for qh in range(2):
                            o = qh * 512
                            # ctx^T[d, q] += V^T P^T ; row 64 = denominator
                            nc.tensor.matmul(
                                ctx_a[0 : HD + 1, o : o + 512],
                                v_sb[:, kc, ha, :],
                                et_a[:, o : o + 512],
                                start=first,
                                stop=last,
                            )
                            nc.tensor.matmul(
                                ctx_b[0 : HD + 1, o : o + 512],
                                v_sb[:, kc, hb, :],
                                et_b[:, o : o + 512],
                                start=first,
                                stop=last,
                            )

                    pv_q = []  # software pipeline: PV(kc-1) after ST(kc)
                    for kc in range(LC):
                        st_a = psA.tile([128, 1024], F32, tag="psA")
                        st_b = psA.tile([128, 1024], F32, tag="psA")
                        for qh in range(2):
                            o = qh * 512
                            # S^T[k, q] = K Q^T for both heads (row-group packed)
                            nc.tensor.matmul(
                                st_a[:, o : o + 512],
                                kt_sb[0:64, hp, kc * 128 : kc * 128 + 128],
                                qt_sb[0:64, hp, o : o + 512],
                                start=True,
                                stop=True,
                            )
                            nc.tensor.matmul(
                                st_b[:, o : o + 512],
                                kt_sb[64:128, hp, kc * 128 : kc * 128 + 128],
                                qt_sb[64:128, hp, o : o + 512],
                                start=True,
                                stop=True,
                            )
                        # P^T = exp(S^T/8 + mask_k)
                        et_a = et_pool.tile([128, 1024], CDT, tag="et")
                        et_b = et_pool.tile([128, 1024], CDT, tag="et")
                        nc.scalar.activation(
                            et_a[:], st_a[:], EXP,
                            bias=mask_sb[:, kc : kc + 1], scale=0.125,
                        )
                        nc.scalar.activation(
                            et_b[:], st_b[:], EXP,
                            bias=mask_sb[:, kc : kc + 1], scale=0.125,
                        )
                        pv_q.append((kc, et_a, et_b))
                        if kc >= 1:
                            emit_pv(*pv_q.pop(0))
                        if kc == 2 and pending is not None:
                            # previous pair's normalize, emitted here so its
                            # matmuls never head-of-line-block the PE
                            pending()
                            pending = None
                    emit_pv(*pv_q.pop(0))
                    # evacuate ctx+denominator to SBUF immediately: frees the
                    # PSUM slot and takes the reciprocal off the PE path
                    ctxu_a = norm_pool.tile([65, 1024], F32, tag="cua")
                    nc.vector.tensor_copy(ctxu_a[:], ctx_a[0:65, :])
                    ctxu_b = norm_pool.tile([65, 1024], F32, tag="cub")
                    nc.vector.tensor_copy(ctxu_b[:], ctx_b[0:65, :])
                    ra = norm_pool.tile([65, 1024], F32, tag="ra")
                    rb = norm_pool.tile([65, 1024], F32, tag="rb")
                    with nc.allow_low_precision(reason="fp32 out"):
                        nc.vector.reciprocal(ra[64:65, :], ctxu_a[64:65, :])
                        nc.vector.reciprocal(rb[64:65, :], ctxu_b[64:65, :])
                    pending = make_normalize(hp, ctxu_a, ctxu_b, ra, rb)
                if pending is not None:
                    pending()
                    pending = None

                # ---- output projection: out[q, o] = ctx Wo^T + bo ----
                for lc in range(LC):
                    ps = psA.tile([128, 1024], F32, tag="psA")
                    for off, width in ((0, 512), (512, 256)):
                        for c in range(NC):
                            nc.tensor.matmul(
                                ps[:, off : off + width],
                                ctxt_sb[:, c, lc * 128 : lc * 128 + 128],
                                wo_sb[:, c, off : off + width],
                                start=(c == 0),
                                stop=False,
                            )
                        nc.tensor.matmul(  # + bo
                            ps[:, off : off + width],
                            ones[0:1, 0:128],
                            bo_sb[0:1, off : off + width],
                            start=False,
                            stop=True,
                        )
                    o_sb = out_pool.tile([128, H], F32, tag="outp")
                    nc.vector.tensor_copy(o_sb[:], ps[:, 0:H])
                    nc.sync.dma_start(out_e[lc * 128 : lc * 128 + 128, :], o_sb[:])

    nc.finalize()
    nc.m = get_hw_module(nc.m)
    return nc


_NC_CACHE = {}


def _get_nc(compute_rounded: bool = True):
    if compute_rounded not in _NC_CACHE:
        _NC_CACHE[compute_rounded] = build_bass(compute_rounded)
    return _NC_CACHE[compute_rounded]


def make_in_maps(inputs):
    f = lambda a: np.ascontiguousarray(np.asarray(a, dtype=np.float32))  # noqa: E731
    hs = f(inputs["hidden_states"])
    mask = f(inputs["attention_mask"]).reshape(B, L)
    shared = {
        "wqt": f(np.asarray(inputs["Wq"]).T),
        "wkt": f(np.asarray(inputs["Wk"]).T),
        "wvt": f(np.asarray(inputs["Wv"]).T),
        "wot": f(np.asarray(inputs["Wo"]).T),
        "bq": f(inputs["bq"]),
        "bk": f(inputs["bk"]),
        "bv": f(inputs["bv"]),
        "bo": f(inputs["bo"]),
    }
    return [
        {"xt": f(hs[b].T), "mask": mask[b], **shared}
        for b in range(B)
    ]


def run_spmd(inputs, trace=False, compute_rounded=True):
    nc = _get_nc(compute_rounded)
    res = run_bass_kernel_spmd(nc, make_in_maps(inputs), list(range(B)), trace=trace)
    out = np.stack([res.results[b]["out"] for b in range(B)]).astype(np.float32)
    return out, res


def kernel(**inputs) -> np.ndarray:
    out, _ = run_spmd(inputs, trace=False)
    return out


# revision 10
# speedup vs baseline: 1.2512x; 1.1777x over previous
"""BERT self-attention on 8 Trainium2 NeuronCores.

Sharding: data-parallel over batch (B=8 -> one batch element per core).
Each core computes full self-attention for its batch element:
  Q/K/V projections, per-head softmax(Q K^T / 8 + mask) V, output proj.

Layout strategy (per core):
  - Host passes xT = x.T [768,1024] and W.T [768,768] so every matmul
    contracts over the partition axis.
  - QT,KT [d, L] and V [L, d] are produced directly by the projections.
  - Attention runs transposed: ST[k,q] = K Q^T per head, so softmax's
    reduction axis (k) lands on partitions: exp via ScalarE with the
    attention mask as per-partition bias (no max subtraction: scores are
    ~N(0,1), |s|<~7, exp is safe in fp32); the denominator comes from a
    ones column appended to V (out row 64); P^T V accumulates ctx^T
    [d, q] which feeds the output projection as lhsT directly.
  - Matmul inputs are float32r (full PE speed at N>=512), accumulation
    and softmax in fp32.
"""

import numpy as np

import concourse.bass as bass  # noqa: F401
import concourse.mybir as mybir
import concourse.tile as tile
from concourse import bacc
from concourse.bass_interp import get_hw_module
from concourse.bass_utils import run_bass_kernel_spmd

B, L, H = 8, 1024, 768
NH, HD = 12, 64
NC = H // 128          # 6 chunks of hidden dim
LC = L // 128          # 8 chunks of sequence dim
F32 = mybir.dt.float32
EXP = mybir.ActivationFunctionType.Exp


def build_bass(compute_rounded: bool = True):
    CDT = mybir.dt.float32r if compute_rounded else F32

    nc = bacc.Bacc("TRN2", debug=False, num_devices=8)

    xt_e = nc.declare_dram_parameter("xt", [H, L], CDT, isOutput=False)
    wqt_e = nc.declare_dram_parameter("wqt", [H, H], CDT, isOutput=False)
    wkt_e = nc.declare_dram_parameter("wkt", [H, H], CDT, isOutput=False)
    wvt_e = nc.declare_dram_parameter("wvt", [H, H], CDT, isOutput=False)
    wot_e = nc.declare_dram_parameter("wot", [H, H], CDT, isOutput=False)
    bq_e = nc.declare_dram_parameter("bq", [H], F32, isOutput=False)
    bk_e = nc.declare_dram_parameter("bk", [H], F32, isOutput=False)
    bv_e = nc.declare_dram_parameter("bv", [H], CDT, isOutput=False)
    bo_e = nc.declare_dram_parameter("bo", [H], CDT, isOutput=False)
    mask_e = nc.declare_dram_parameter("mask", [L], F32, isOutput=False)
    out_e = nc.declare_dram_parameter("out", [L, H], F32, isOutput=True)

    with tile.TileContext(nc) as tc:
        with (
            tc.tile_pool(name="small", bufs=1) as small,
            tc.tile_pool(name="acts", bufs=1) as acts,
            tc.tile_pool(name="outp", bufs=2) as out_pool,
            tc.tile_pool(name="psA", bufs=2, space="PSUM") as psA,
            tc.tile_pool(name="psB", bufs=2, space="PSUM") as psB,
        ):
            # ---- constants / small tensors ----
            mask_sb = small.tile([128, LC], F32)
            nc.sync.dma_start(mask_sb[:], mask_e[:].rearrange("(c p) -> p c", p=128))
            bq_sb = small.tile([128, NC], F32)
            nc.sync.dma_start(bq_sb[:], bq_e[:].rearrange("(c p) -> p c", p=128))
            bk_sb = small.tile([128, NC], F32)
            nc.sync.dma_start(bk_sb[:], bk_e[:].rearrange("(c p) -> p c", p=128))
            bv_sb = small.tile([1, H], CDT)
            nc.sync.dma_start(bv_sb[:], bv_e[None, :])
            bo_sb = small.tile([1, H], CDT)
            nc.sync.dma_start(bo_sb[:], bo_e[None, :])
            ones32 = small.tile([128, 128], F32)
            nc.vector.memset(ones32[:], 1.0)
            ones = small.tile([128, 128], CDT)
            nc.vector.tensor_copy(ones[:], ones32[:])

            BF = mybir.dt.bfloat16
            qt_sb = acts.tile([128, NC, L], BF)
            kt_sb = acts.tile([128, NH, L], BF)  # per-head K^T, other 64 rows zero
            nc.gpsimd.memset(kt_sb[:], 0.0)
            v_sb = acts.tile([128, LC, NH, HD + 1], CDT)  # [..., 64] = ones col
            ctxt_sb = acts.tile([128, NC, L], CDT)

            nc.vector.tensor_copy(
                v_sb[:, :, :, HD],
                ones32[:, 0 : LC * NH].rearrange("p (a b) -> p a b", a=LC),
            )

            # =========== projection phase (xt + wv/wk/wq scoped) ===========
            with (
                tc.tile_pool(name="xt", bufs=1) as xt_pool,
                tc.tile_pool(name="w1", bufs=2) as w1,
            ):
                xt_sb = xt_pool.tile([128, NC, L], CDT)
                for c in range(NC):
                    nc.sync.dma_start(
                        xt_sb[:, c, :],
                        xt_e[:].rearrange("(c p) q -> p c q", p=128)[:, c, :],
                    )

                # ---- V projection: V[l, d] = x Wv^T + bv (natural layout)
                wv_sb = w1.tile([128, NC, H], CDT, tag="w")
                for c in range(NC):
                    nc.sync.dma_start(
                        wv_sb[:, c, :],
                        wvt_e[:].rearrange("(c p) d -> p c d", p=128)[:, c, :],
                    )
                for lc in range(LC):
                    ps = psA.tile([128, 1024], F32, tag="psA")
                    for off, width in ((0, 512), (512, 256)):
                        for kc in range(NC):
                            nc.tensor.matmul(
                                ps[:, off : off + width],
                                xt_sb[:, kc, lc * 128 : lc * 128 + 128],
                                wv_sb[:, kc, off : off + width],
                                start=(kc == 0),
                                stop=False,
                            )
                        nc.tensor.matmul(  # + bv (ones row x bias row)
                            ps[:, off : off + width],
                            ones[0:1, 0:128],
                            bv_sb[0:1, off : off + width],
                            start=False,
                            stop=True,
                        )
                    nc.vector.tensor_copy(
                        v_sb[:, lc, :, 0:HD],
                        ps[:, 0:H].rearrange("p (h d) -> p h d", d=HD),
                    )

                # ---- K^T then Q^T projections: out[d, q] = W x^T + b
                for w_e, b_sb, dst in ((wkt_e, bk_sb, kt_sb), (wqt_e, bq_sb, qt_sb)):
                    w_sb = w1.tile([128, NC, H], CDT, tag="w")
                    for c in range(NC):
                        nc.sync.dma_start(
                            w_sb[:, c, :],
                            w_e[:].rearrange("(c p) d -> p c d", p=128)[:, c, :],
                        )
                    for dc in range(NC):
                        ps = psA.tile([128, 1024], F32, tag="psA")
                        for qh in range(2):
                            o = qh * 512
                            for kc in range(NC):
                                nc.tensor.matmul(
                                    ps[:, o : o + 512],
                                    w_sb[:, kc, dc * 128 : dc * 128 + 128],
                                    xt_sb[:, kc, o : o + 512],
                                    start=(kc == 0),
                                    stop=(kc == NC - 1),
                                )
                        if dst is qt_sb:
                            nc.vector.tensor_scalar_add(
                                dst[:, dc, :], ps[:, :], b_sb[:, dc : dc + 1]
                            )
                        else:
                            nc.vector.tensor_scalar_add(
                                kt_sb[0:64, 2 * dc, :], ps[0:64, :],
                                b_sb[0:64, dc : dc + 1],
                            )
                            nc.vector.tensor_scalar_add(
                                kt_sb[64:128, 2 * dc + 1, :], ps[64:128, :],
                                b_sb[64:128, dc : dc + 1],
                            )

            # =========== attention + output projection ===========
            with (
                tc.tile_pool(name="w2", bufs=1) as w2,
                tc.tile_pool(name="et", bufs=6) as et_pool,
                tc.tile_pool(name="norm", bufs=1) as norm_pool,
            ):
                wo_sb = w2.tile([128, NC, H], CDT)
                for c in range(NC):
                    nc.sync.dma_start(
                        wo_sb[:, c, :],
                        wot_e[:].rearrange("(c p) d -> p c d", p=128)[:, c, :],
                    )

                def make_normalize(hp, ctxu_a, ctxu_b, ra, rb):
                    def emit():
                        # broadcast 1/denom over 64 partitions via f32r matmul,
                        # then scale ctx^T and store to ctxt_sb
                        for recip, ctxu, btag in ((ra, ctxu_a, "bca"),
                                                  (rb, ctxu_b, "bcb")):
                            bc = psA.tile([64, 1024], F32, tag="psA")
                            for o in (0, 512):
                                nc.tensor.matmul(
                                    bc[:, o : o + 512],
                                    ones[64:65, 0:64],
                                    recip[64:65, o : o + 512],
                                    start=True,
                                    stop=True,
                                )
                            bc_sb = norm_pool.tile([64, 1024], F32, tag=btag)
                            nc.vector.tensor_copy(bc_sb[:], bc[:])
                            if btag == "bca":
                                nc.gpsimd.tensor_tensor(
                                    ctxt_sb[0:64, hp, :], ctxu[0:64, :], bc_sb[:],
                                    mybir.AluOpType.mult,
                                )
                            else:
                                tmp_o = norm_pool.tile([64, 1024], CDT, tag="tmp")
                                nc.gpsimd.tensor_tensor(
                                    tmp_o[:], ctxu[0:64, :], bc_sb[:],
                                    mybir.AluOpType.mult,
                                )
                                # lift odd head to partitions 64:128 (DMA can
                                # cross partitions; DVE cannot)
                                nc.sync.dma_start(ctxt_sb[64:128, hp, :], tmp_o[:])
                    return emit

                pending = None
                for hp in range(NH // 2):
                    ha, hb = 2 * hp, 2 * hp + 1
                    # per head: rows 0:64 = ctx^T, row 64 = softmax denominator
                    ctx_a = psB.tile([128, 1024], F32, tag="psB")
                    ctx_b = psB.tile([128, 1024], F32, tag="psB")

                    def emit_pv(kc, et_a, et_b, ctx_a=ctx_a, ctx_b=ctx_b,
                                ha=ha, hb=hb):
                        first, last = kc == 0, kc == LC - 1
                        for qh in range(2):
                            o = qh * 512
                            # ctx^T[d, q] += V^T P^T ; row 64 = denominator
                            nc.tensor.matmul(
                                ctx_a[0 : HD + 1, o : o + 512],
                                v_sb[:, kc, ha, :],
                                et_a[:, o : o + 512],
                                start=first,
                                stop=last,
                            )
                            nc.tensor.matmul(
                                ctx_b[0 : HD + 1, o : o + 512],
                                v_sb[:, kc, hb, :],
                                et_b[:, o : o + 512],
                                start=first,
                                stop=last,
                            )

                    pv_q = []  # software pipeline: PV(kc-1) after ST(kc)
                    for kc in range(LC):
                        st_a = psA.tile([128, 1024], F32, tag="psA")
                        st_b = psA.tile([128, 1024], F32, tag="psA")
                        for qh in range(2):
                            o = qh * 512
                            # S^T[k, q] = K Q^T for both heads (row-group packed)
                            nc.tensor.matmul(
                                st_a[:, o : o + 512],
                                kt_sb[:, ha, kc * 128 : kc * 128 + 128],
                                qt_sb[:, hp, o : o + 512],
                                start=True,
                                stop=True,
                            )
                            nc.tensor.matmul(
                                st_b[:, o : o + 512],
                                kt_sb[:, hb, kc * 128 : kc * 128 + 128],
                                qt_sb[:, hp, o : o + 512],
                                start=True,
                                stop=True,
                            )
                        # P^T = exp(S^T/8 + mask_k)
                        et_a = et_pool.tile([128, 1024], CDT, tag="et")
                        et_b = et_pool.tile([128, 1024], CDT, tag="et")
                        nc.scalar.activation(
                            et_a[:], st_a[:], EXP,
                            bias=mask_sb[:, kc : kc + 1], scale=0.125,
                        )
                        nc.scalar.activation(
                            et_b[:], st_b[:], EXP,
                            bias=mask_sb[:, kc : kc + 1], scale=0.125,
                        )
                        pv_q.append((kc, et_a, et_b))
                        if kc >= 1:
                            emit_pv(*pv_q.pop(0))
                        if kc == 2 and pending is not None:
                            # previous pair's normalize, emitted here so its
                            # matmuls never head-of-line-block the PE
                            pending()
                            pending = None
                    emit_pv(*pv_q.pop(0))
                    # evacuate ctx+denominator to SBUF immediately: frees the
                    # PSUM slot and takes the reciprocal off the PE path
                    ctxu_a = norm_pool.tile([65, 1024], F32, tag="cua")
                    nc.vector.tensor_copy(ctxu_a[:], ctx_a[0:65, :])
                    ctxu_b = norm_pool.tile([65, 1024], F32, tag="cub")
                    nc.vector.tensor_copy(ctxu_b[:], ctx_b[0:65, :])
                    ra32 = norm_pool.tile([65, 1024], F32, tag="ra32")
                    rb32 = norm_pool.tile([65, 1024], F32, tag="rb32")
                    nc.vector.reciprocal(ra32[64:65, :], ctxu_a[64:65, :])
                    nc.vector.reciprocal(rb32[64:65, :], ctxu_b[64:65, :])
                    ra = norm_pool.tile([65, 1024], CDT, tag="ra")
                    rb = norm_pool.tile([65, 1024], CDT, tag="rb")
                    nc.vector.tensor_copy(ra[64:65, :], ra32[64:65, :])
                    nc.vector.tensor_copy(rb[64:65, :], rb32[64:65, :])
                    pending = make_normalize(hp, ctxu_a, ctxu_b, ra, rb)
                if pending is not None:
                    pending()
                    pending = None

                # ---- output projection: out[q, o] = ctx Wo^T + bo ----
                for lc in range(LC):
                    ps = psA.tile([128, 1024], F32, tag="psA")
                    for off, width in ((0, 512), (512, 256)):
                        for c in range(NC):
                            nc.tensor.matmul(
                                ps[:, off : off + width],
                                ctxt_sb[:, c, lc * 128 : lc * 128 + 128],
                                wo_sb[:, c, off : off + width],
                                start=(c == 0),
                                stop=False,
                            )
                        nc.tensor.matmul(  # + bo
                            ps[:, off : off + width],
                            ones[0:1, 0:128],
                            bo_sb[0:1, off : off + width],
                            start=False,
                            stop=True,
                        )
                    o_sb = out_pool.tile([128, H], F32, tag="outp")
                    nc.vector.tensor_copy(o_sb[:], ps[:, 0:H])
                    nc.sync.dma_start(out_e[lc * 128 : lc * 128 + 128, :], o_sb[:])

    nc.finalize()
    nc.m = get_hw_module(nc.m)
    return nc


_NC_CACHE = {}


def _get_nc(compute_rounded: bool = True):
    if compute_rounded not in _NC_CACHE:
        _NC_CACHE[compute_rounded] = build_bass(compute_rounded)
    return _NC_CACHE[compute_rounded]


def make_in_maps(inputs):
    f = lambda a: np.ascontiguousarray(np.asarray(a, dtype=np.float32))  # noqa: E731
    hs = f(inputs["hidden_states"])
    mask = f(inputs["attention_mask"]).reshape(B, L)
    shared = {
        "wqt": f(np.asarray(inputs["Wq"]).T),
        "wkt": f(np.asarray(inputs["Wk"]).T),
        "wvt": f(np.asarray(inputs["Wv"]).T),
        "wot": f(np.asarray(inputs["Wo"]).T),
        "bq": f(inputs["bq"]),
        "bk": f(inputs["bk"]),
        "bv": f(inputs["bv"]),
        "bo": f(inputs["bo"]),
    }
    return [
        {"xt": f(hs[b].T), "mask": mask[b], **shared}
        for b in range(B)
    ]


def run_spmd(inputs, trace=False, compute_rounded=True):
    nc = _get_nc(compute_rounded)
    res = run_bass_kernel_spmd(nc, make_in_maps(inputs), list(range(B)), trace=trace)
    out = np.stack([res.results[b]["out"] for b in range(B)]).astype(np.float32)
    return out, res


def kernel(**inputs) -> np.ndarray:
    out, _ = run_spmd(inputs, trace=False)
    return out


# revision 12
# speedup vs baseline: 1.3449x; 1.0749x over previous
"""BERT self-attention on 8 Trainium2 NeuronCores.

Sharding: data-parallel over batch (B=8 -> one batch element per core).
Each core computes full self-attention for its batch element:
  Q/K/V projections, per-head softmax(Q K^T / 8 + mask) V, output proj.

Layout strategy (per core):
  - Host passes xT = x.T [768,1024] and W.T [768,768] so every matmul
    contracts over the partition axis.
  - QT,KT [d, L] and V [L, d] are produced directly by the projections.
  - Attention runs transposed: ST[k,q] = K Q^T per head, so softmax's
    reduction axis (k) lands on partitions: exp via ScalarE with the
    attention mask as per-partition bias (no max subtraction: scores are
    ~N(0,1), |s|<~7, exp is safe in fp32); the denominator comes from a
    ones column appended to V (out row 64); P^T V accumulates ctx^T
    [d, q] which feeds the output projection as lhsT directly.
  - Matmul inputs are float32r (full PE speed at N>=512), accumulation
    and softmax in fp32.
"""

import numpy as np

import concourse.bass as bass  # noqa: F401
import concourse.mybir as mybir
import concourse.tile as tile
from concourse import bacc
from concourse.bass_interp import get_hw_module
from concourse.bass_utils import run_bass_kernel_spmd

B, L, H = 8, 1024, 768
NH, HD = 12, 64
NC = H // 128          # 6 chunks of hidden dim
LC = L // 128          # 8 chunks of sequence dim
F32 = mybir.dt.float32
EXP = mybir.ActivationFunctionType.Exp


def build_bass(compute_rounded: bool = True):
    CDT = mybir.dt.float32r if compute_rounded else F32

    nc = bacc.Bacc("TRN2", debug=False, num_devices=8)

    xt_e = nc.declare_dram_parameter("xt", [H, L], CDT, isOutput=False)
    wqt_e = nc.declare_dram_parameter("wqt", [H, H], CDT, isOutput=False)
    wkt_e = nc.declare_dram_parameter("wkt", [H, H], CDT, isOutput=False)
    wvt_e = nc.declare_dram_parameter("wvt", [H, H], CDT, isOutput=False)
    wot_e = nc.declare_dram_parameter("wot", [H, H], CDT, isOutput=False)
    bq_e = nc.declare_dram_parameter("bq", [H], F32, isOutput=False)
    bk_e = nc.declare_dram_parameter("bk", [H], F32, isOutput=False)
    bv_e = nc.declare_dram_parameter("bv", [H], CDT, isOutput=False)
    bo_e = nc.declare_dram_parameter("bo", [H], CDT, isOutput=False)
    mask_e = nc.declare_dram_parameter("mask", [L], F32, isOutput=False)
    out_e = nc.declare_dram_parameter("out", [L, H], F32, isOutput=True)

    with tile.TileContext(nc) as tc:
        with (
            tc.tile_pool(name="small", bufs=1) as small,
            tc.tile_pool(name="acts", bufs=1) as acts,
            tc.tile_pool(name="outp", bufs=2) as out_pool,
            tc.tile_pool(name="psA", bufs=2, space="PSUM") as psA,
            tc.tile_pool(name="psB", bufs=2, space="PSUM") as psB,
        ):
            # ---- constants / small tensors ----
            mask_sb = small.tile([128, LC], F32)
            nc.sync.dma_start(mask_sb[:], mask_e[:].rearrange("(c p) -> p c", p=128))
            bq_sb = small.tile([128, NC], F32)
            nc.sync.dma_start(bq_sb[:], bq_e[:].rearrange("(c p) -> p c", p=128))
            bk_sb = small.tile([128, NC], F32)
            nc.sync.dma_start(bk_sb[:], bk_e[:].rearrange("(c p) -> p c", p=128))
            bv_sb = small.tile([1, H], CDT)
            nc.sync.dma_start(bv_sb[:], bv_e[None, :])
            bo_sb = small.tile([1, H], CDT)
            nc.sync.dma_start(bo_sb[:], bo_e[None, :])
            ones32 = small.tile([128, 128], F32)
            nc.vector.memset(ones32[:], 1.0)
            ones = small.tile([128, 128], CDT)
            nc.vector.tensor_copy(ones[:], ones32[:])

            BF = mybir.dt.bfloat16
            qt_sb = acts.tile([128, NC, L], BF)
            kt_sb = acts.tile([128, NH, L], BF)  # per-head K^T, other 64 rows zero
            nc.gpsimd.memset(kt_sb[:], 0.0)
            v_sb = acts.tile([128, LC, NH, HD + 1], CDT)  # [..., 64] = ones col
            ctxt_sb = acts.tile([128, NC, L], CDT)

            nc.vector.tensor_copy(
                v_sb[:, :, :, HD],
                ones32[:, 0 : LC * NH].rearrange("p (a b) -> p a b", a=LC),
            )

            # =========== projection phase (xt + wv/wk/wq scoped) ===========
            with (
                tc.tile_pool(name="xt", bufs=1) as xt_pool,
                tc.tile_pool(name="w1", bufs=2) as w1,
            ):
                xt_sb = xt_pool.tile([128, NC, L], CDT)
                for c in range(NC):
                    nc.sync.dma_start(
                        xt_sb[:, c, :],
                        xt_e[:].rearrange("(c p) q -> p c q", p=128)[:, c, :],
                    )

                # ---- V projection: V[l, d] = x Wv^T + bv (natural layout)
                wv_sb = w1.tile([128, NC, H], CDT, tag="w")
                for c in range(NC):
                    nc.sync.dma_start(
                        wv_sb[:, c, :],
                        wvt_e[:].rearrange("(c p) d -> p c d", p=128)[:, c, :],
                    )
                for lc in range(LC):
                    ps = psA.tile([128, 1024], F32, tag="psA")
                    for off, width in ((0, 512), (512, 256)):
                        for kc in range(NC):
                            nc.tensor.matmul(
                                ps[:, off : off + width],
                                xt_sb[:, kc, lc * 128 : lc * 128 + 128],
                                wv_sb[:, kc, off : off + width],
                                start=(kc == 0),
                                stop=False,
                            )
                        nc.tensor.matmul(  # + bv (ones row x bias row)
                            ps[:, off : off + width],
                            ones[0:1, 0:128],
                            bv_sb[0:1, off : off + width],
                            start=False,
                            stop=True,
                        )
                    nc.vector.tensor_copy(
                        v_sb[:, lc, :, 0:HD],
                        ps[:, 0:H].rearrange("p (h d) -> p h d", d=HD),
                    )

                # ---- K^T then Q^T projections: out[d, q] = W x^T + b
                for w_e, b_sb, dst in ((wkt_e, bk_sb, kt_sb), (wqt_e, bq_sb, qt_sb)):
                    w_sb = w1.tile([128, NC, H], CDT, tag="w")
                    for c in range(NC):
                        nc.sync.dma_start(
                            w_sb[:, c, :],
                            w_e[:].rearrange("(c p) d -> p c d", p=128)[:, c, :],
                        )
                    for dc in range(NC):
                        ps = psA.tile([128, 1024], F32, tag="psA")
                        for qh in range(2):
                            o = qh * 512
                            for kc in range(NC):
                                nc.tensor.matmul(
                                    ps[:, o : o + 512],
                                    w_sb[:, kc, dc * 128 : dc * 128 + 128],
                                    xt_sb[:, kc, o : o + 512],
                                    start=(kc == 0),
                                    stop=(kc == NC - 1),
                                )
                        if dst is qt_sb:
                            nc.vector.tensor_scalar_add(
                                dst[:, dc, :], ps[:, :], b_sb[:, dc : dc + 1]
                            )
                        else:
                            nc.vector.tensor_scalar_add(
                                kt_sb[0:64, 2 * dc, :], ps[0:64, :],
                                b_sb[0:64, dc : dc + 1],
                            )
                            nc.vector.tensor_scalar_add(
                                kt_sb[64:128, 2 * dc + 1, :], ps[64:128, :],
                                b_sb[64:128, dc : dc + 1],
                            )

            # =========== attention + output projection ===========
            with (
                tc.tile_pool(name="w2", bufs=1) as w2,
                tc.tile_pool(name="et", bufs=6) as et_pool,
                tc.tile_pool(name="norm", bufs=1) as norm_pool,
            ):
                wo_sb = w2.tile([128, NC, H], CDT)
                for c in range(NC):
                    nc.sync.dma_start(
                        wo_sb[:, c, :],
                        wot_e[:].rearrange("(c p) d -> p c d", p=128)[:, c, :],
                    )

                def make_normalize(hp, ctxu_a, ctxu_b, ra, rb):
                    def emit():
                        # broadcast 1/denom over 64 partitions via f32r matmul,
                        # then scale ctx^T and store to ctxt_sb
                        for recip, ctxu, btag in ((ra, ctxu_a, "bca"),
                                                  (rb, ctxu_b, "bcb")):
                            bc = psA.tile([64, 1024], F32, tag="psA")
                            for o in (0, 512):
                                nc.tensor.matmul(
                                    bc[:, o : o + 512],
                                    ones[64:65, 0:64],
                                    recip[64:65, o : o + 512],
                                    start=True,
                                    stop=True,
                                )
                            bc_sb = norm_pool.tile([64, 1024], F32, tag=btag)
                            nc.vector.tensor_copy(bc_sb[:], bc[:])
                            if btag == "bca":
                                nc.gpsimd.tensor_tensor(
                                    ctxt_sb[0:64, hp, :], ctxu[0:64, :], bc_sb[:],
                                    mybir.AluOpType.mult,
                                )
                            else:
                                tmp_o = norm_pool.tile([64, 1024], CDT, tag="tmp")
                                nc.gpsimd.tensor_tensor(
                                    tmp_o[:], ctxu[0:64, :], bc_sb[:],
                                    mybir.AluOpType.mult,
                                )
                                # lift odd head to partitions 64:128 (DMA can
                                # cross partitions; DVE cannot)
                                nc.sync.dma_start(ctxt_sb[64:128, hp, :], tmp_o[:])
                    return emit

                pending = None
                for hp in range(NH // 2):
                    ha, hb = 2 * hp, 2 * hp + 1
                    # per head: rows 0:64 = ctx^T, row 64 = softmax denominator
                    ctx_a = psB.tile([128, 1024], F32, tag="psB")
                    ctx_b = psB.tile([128, 1024], F32, tag="psB")

                    def emit_pv(kc, et_a, et_b, ctx_a=ctx_a, ctx_b=ctx_b,
                                ha=ha, hb=hb):
                        first, last = kc == 0, kc == LC - 1
                        for qh in range(2):
                            o = qh * 512
                            # ctx^T[d, q] += V^T P^T ; row 64 = denominator
                            nc.tensor.matmul(
                                ctx_a[0 : HD + 1, o : o + 512],
                                v_sb[:, kc, ha, :],
                                et_a[:, o : o + 512],
                                start=first,
                                stop=last,
                            )
                            nc.tensor.matmul(
                                ctx_b[0 : HD + 1, o : o + 512],
                                v_sb[:, kc, hb, :],
                                et_b[:, o : o + 512],
                                start=first,
                                stop=last,
                            )

                    pv_q = []  # software pipeline: PV(kc-1) after ST(kc)
                    for kc in range(LC):
                        st_a = psA.tile([128, 1024], F32, tag="psA")
                        st_b = psA.tile([128, 1024], F32, tag="psA")
                        for qh in range(2):
                            o = qh * 512
                            # S^T[k, q] = K Q^T for both heads (row-group packed)
                            nc.tensor.matmul(
                                st_a[:, o : o + 512],
                                kt_sb[:, ha, kc * 128 : kc * 128 + 128],
                                qt_sb[:, hp, o : o + 512],
                                start=True,
                                stop=True,
                            )
                            nc.tensor.matmul(
                                st_b[:, o : o + 512],
                                kt_sb[:, hb, kc * 128 : kc * 128 + 128],
                                qt_sb[:, hp, o : o + 512],
                                start=True,
                                stop=True,
                            )
                        # P^T = exp(S^T/8 + mask_k)
                        et_a = et_pool.tile([128, 1024], CDT, tag="et")
                        et_b = et_pool.tile([128, 1024], CDT, tag="et")
                        nc.scalar.activation(
                            et_a[:], st_a[:], EXP,
                            bias=mask_sb[:, kc : kc + 1], scale=0.125,
                        )
                        nc.scalar.activation(
                            et_b[:], st_b[:], EXP,
                            bias=mask_sb[:, kc : kc + 1], scale=0.125,
                        )
                        pv_q.append((kc, et_a, et_b))
                        if kc >= 1:
                            emit_pv(*pv_q.pop(0))
                        if kc == 3 and pending is not None:
                            # previous pair's normalize, emitted here so its
                            # matmuls never head-of-line-block the PE
                            pending()
                            pending = None
                    emit_pv(*pv_q.pop(0))
                    # evacuate ctx+denominator to SBUF immediately: frees the
                    # PSUM slot and takes the reciprocal off the PE path
                    ctxu_a = norm_pool.tile([65, 1024], F32, tag="cua")
                    nc.vector.tensor_copy(ctxu_a[:], ctx_a[0:65, :])
                    ctxu_b = norm_pool.tile([65, 1024], F32, tag="cub")
                    nc.vector.tensor_copy(ctxu_b[:], ctx_b[0:65, :])
                    # 1/d = exp(-ln d) on ScalarE: Log+Exp share one ACT
                    # table set, ~1e-5 rel err, and it keeps the reciprocal
                    # off the (slow, 1-lane) DVE path entirely
                    LOG = mybir.ActivationFunctionType.Ln
                    lna = norm_pool.tile([65, 1024], F32, tag="lna")
                    nc.scalar.activation(lna[64:65, :], ctxu_a[64:65, :], LOG)
                    lnb = norm_pool.tile([65, 1024], F32, tag="lnb")
                    nc.scalar.activation(lnb[64:65, :], ctxu_b[64:65, :], LOG)
                    ra = norm_pool.tile([65, 1024], CDT, tag="ra")
                    rb = norm_pool.tile([65, 1024], CDT, tag="rb")
                    nc.scalar.activation(ra[64:65, :], lna[64:65, :], EXP, scale=-1.0)
                    nc.scalar.activation(rb[64:65, :], lnb[64:65, :], EXP, scale=-1.0)
                    pending = make_normalize(hp, ctxu_a, ctxu_b, ra, rb)
                if pending is not None:
                    pending()
                    pending = None

                # ---- output projection: out[q, o] = ctx Wo^T + bo ----
                for lc in range(LC):
                    ps = psA.tile([128, 1024], F32, tag="psA")
                    for off, width in ((0, 512), (512, 256)):
                        for c in range(NC):
                            nc.tensor.matmul(
                                ps[:, off : off + width],
                                ctxt_sb[:, c, lc * 128 : lc * 128 + 128],
                                wo_sb[:, c, off : off + width],
                                start=(c == 0),
                                stop=False,
                            )
                        nc.tensor.matmul(  # + bo
                            ps[:, off : off + width],
                            ones[0:1, 0:128],
                            bo_sb[0:1, off : off + width],
                            start=False,
                            stop=True,
                        )
                    o_sb = out_pool.tile([128, H], F32, tag="outp")
                    nc.vector.tensor_copy(o_sb[:], ps[:, 0:H])
                    nc.sync.dma_start(out_e[lc * 128 : lc * 128 + 128, :], o_sb[:])

    nc.finalize()
    nc.m = get_hw_module(nc.m)
    return nc


_NC_CACHE = {}


def _get_nc(compute_rounded: bool = True):
    if compute_rounded not in _NC_CACHE:
        _NC_CACHE[compute_rounded] = build_bass(compute_rounded)
    return _NC_CACHE[compute_rounded]


def make_in_maps(inputs):
    f = lambda a: np.ascontiguousarray(np.asarray(a, dtype=np.float32))  # noqa: E731
    hs = f(inputs["hidden_states"])
    mask = f(inputs["attention_mask"]).reshape(B, L)
    shared = {
        "wqt": f(np.asarray(inputs["Wq"]).T),
        "wkt": f(np.asarray(inputs["Wk"]).T),
        "wvt": f(np.asarray(inputs["Wv"]).T),
        "wot": f(np.asarray(inputs["Wo"]).T),
        "bq": f(inputs["bq"]),
        "bk": f(inputs["bk"]),
        "bv": f(inputs["bv"]),
        "bo": f(inputs["bo"]),
    }
    return [
        {"xt": f(hs[b].T), "mask": mask[b], **shared}
        for b in range(B)
    ]


def run_spmd(inputs, trace=False, compute_rounded=True):
    nc = _get_nc(compute_rounded)
    res = run_bass_kernel_spmd(nc, make_in_maps(inputs), list(range(B)), trace=trace)
    out = np.stack([res.results[b]["out"] for b in range(B)]).astype(np.float32)
    return out, res


def kernel(**inputs) -> np.ndarray:
    out, _ = run_spmd(inputs, trace=False)
    return out


# revision 13
# speedup vs baseline: 1.4456x; 1.0749x over previous
"""BERT self-attention on 8 Trainium2 NeuronCores.

Sharding: data-parallel over batch (B=8 -> one batch element per core).
Each core computes full self-attention for its batch element:
  Q/K/V projections, per-head softmax(Q K^T / 8 + mask) V, output proj.

Layout strategy (per core):
  - Host passes xT = x.T [768,1024] and W.T [768,768] so every matmul
    contracts over the partition axis.
  - QT,KT [d, L] and V [L, d] are produced directly by the projections.
  - Attention runs transposed: ST[k,q] = K Q^T per head, so softmax's
    reduction axis (k) lands on partitions: exp via ScalarE with the
    attention mask as per-partition bias (no max subtraction: scores are
    ~N(0,1), |s|<~7, exp is safe in fp32); the denominator comes from a
    ones column appended to V (out row 64); P^T V accumulates ctx^T
    [d, q] which feeds the output projection as lhsT directly.
  - Matmul inputs are float32r (full PE speed at N>=512), accumulation
    and softmax in fp32.
"""

import numpy as np

import concourse.bass as bass  # noqa: F401
import concourse.mybir as mybir
import concourse.tile as tile
from concourse import bacc
from concourse.bass_interp import get_hw_module
from concourse.bass_utils import run_bass_kernel_spmd

B, L, H = 8, 1024, 768
NH, HD = 12, 64
NC = H // 128          # 6 chunks of hidden dim
LC = L // 128          # 8 chunks of sequence dim
F32 = mybir.dt.float32
EXP = mybir.ActivationFunctionType.Exp


def build_bass(compute_rounded: bool = True):
    CDT = mybir.dt.float32r if compute_rounded else F32

    nc = bacc.Bacc("TRN2", debug=False, num_devices=8)

    # The kernel uses Exp (softmax) and Ln (reciprocal via exp(-ln d)).
    # Walrus's table-load pass would alternate exp_and_others /
    # natural_log sets (a ~1.3us ACT table DMA per switch, dozens per
    # kernel). Make the combined natural_log_exp_and_others set the only
    # provider of Exp/Ln so a single table load serves the whole kernel.
    from concourse.hw_specs import get_activation_tables

    _tabs = get_activation_tables(nc.m.arch)
    _E = mybir.ActivationFunctionType.Exp
    _L = mybir.ActivationFunctionType.Ln
    if "natural_log_exp_and_others" in _tabs:
        for _name, _fns in _tabs.items():
            if _name != "natural_log_exp_and_others":
                _fns.discard(_E)
                _fns.discard(_L)

    xt_e = nc.declare_dram_parameter("xt", [H, L], CDT, isOutput=False)
    wqt_e = nc.declare_dram_parameter("wqt", [H, H], CDT, isOutput=False)
    wkt_e = nc.declare_dram_parameter("wkt", [H, H], CDT, isOutput=False)
    wvt_e = nc.declare_dram_parameter("wvt", [H, H], CDT, isOutput=False)
    wot_e = nc.declare_dram_parameter("wot", [H, H], CDT, isOutput=False)
    bq_e = nc.declare_dram_parameter("bq", [H], F32, isOutput=False)
    bk_e = nc.declare_dram_parameter("bk", [H], F32, isOutput=False)
    bv_e = nc.declare_dram_parameter("bv", [H], CDT, isOutput=False)
    bo_e = nc.declare_dram_parameter("bo", [H], CDT, isOutput=False)
    mask_e = nc.declare_dram_parameter("mask", [L], F32, isOutput=False)
    out_e = nc.declare_dram_parameter("out", [L, H], F32, isOutput=True)

    with tile.TileContext(nc) as tc:
        with (
            tc.tile_pool(name="small", bufs=1) as small,
            tc.tile_pool(name="acts", bufs=1) as acts,
            tc.tile_pool(name="outp", bufs=2) as out_pool,
            tc.tile_pool(name="psA", bufs=2, space="PSUM") as psA,
            tc.tile_pool(name="psB", bufs=2, space="PSUM") as psB,
        ):
            # ---- constants / small tensors ----
            mask_sb = small.tile([128, LC], F32)
            nc.sync.dma_start(mask_sb[:], mask_e[:].rearrange("(c p) -> p c", p=128))
            bq_sb = small.tile([128, NC], F32)
            nc.sync.dma_start(bq_sb[:], bq_e[:].rearrange("(c p) -> p c", p=128))
            bk_sb = small.tile([128, NC], F32)
            nc.sync.dma_start(bk_sb[:], bk_e[:].rearrange("(c p) -> p c", p=128))
            bv_sb = small.tile([1, H], CDT)
            nc.sync.dma_start(bv_sb[:], bv_e[None, :])
            bo_sb = small.tile([1, H], CDT)
            nc.sync.dma_start(bo_sb[:], bo_e[None, :])
            ones32 = small.tile([128, 128], F32)
            nc.vector.memset(ones32[:], 1.0)
            ones = small.tile([128, 128], CDT)
            nc.vector.tensor_copy(ones[:], ones32[:])

            BF = mybir.dt.bfloat16
            qt_sb = acts.tile([128, NC, L], BF)
            kt_sb = acts.tile([128, NH, L], BF)  # per-head K^T, other 64 rows zero
            nc.gpsimd.memset(kt_sb[:], 0.0)
            v_sb = acts.tile([128, LC, NH, HD + 1], CDT)  # [..., 64] = ones col
            ctxt_sb = acts.tile([128, NC, L], CDT)

            nc.vector.tensor_copy(
                v_sb[:, :, :, HD],
                ones32[:, 0 : LC * NH].rearrange("p (a b) -> p a b", a=LC),
            )

            # =========== projection phase (xt + wv/wk/wq scoped) ===========
            with (
                tc.tile_pool(name="xt", bufs=1) as xt_pool,
                tc.tile_pool(name="w1", bufs=2) as w1,
            ):
                xt_sb = xt_pool.tile([128, NC, L], CDT)
                for c in range(NC):
                    nc.sync.dma_start(
                        xt_sb[:, c, :],
                        xt_e[:].rearrange("(c p) q -> p c q", p=128)[:, c, :],
                    )

                # ---- V projection: V[l, d] = x Wv^T + bv (natural layout)
                wv_sb = w1.tile([128, NC, H], CDT, tag="w")
                for c in range(NC):
                    nc.sync.dma_start(
                        wv_sb[:, c, :],
                        wvt_e[:].rearrange("(c p) d -> p c d", p=128)[:, c, :],
                    )
                for lc in range(LC):
                    ps = psA.tile([128, 1024], F32, tag="psA")
                    for off, width in ((0, 512), (512, 256)):
                        for kc in range(NC):
                            nc.tensor.matmul(
                                ps[:, off : off + width],
                                xt_sb[:, kc, lc * 128 : lc * 128 + 128],
                                wv_sb[:, kc, off : off + width],
                                start=(kc == 0),
                                stop=False,
                            )
                        nc.tensor.matmul(  # + bv (ones row x bias row)
                            ps[:, off : off + width],
                            ones[0:1, 0:128],
                            bv_sb[0:1, off : off + width],
                            start=False,
                            stop=True,
                        )
                    nc.vector.tensor_copy(
                        v_sb[:, lc, :, 0:HD],
                        ps[:, 0:H].rearrange("p (h d) -> p h d", d=HD),
                    )

                # ---- K^T then Q^T projections: out[d, q] = W x^T + b
                for w_e, b_sb, dst in ((wkt_e, bk_sb, kt_sb), (wqt_e, bq_sb, qt_sb)):
                    w_sb = w1.tile([128, NC, H], CDT, tag="w")
                    for c in range(NC):
                        nc.sync.dma_start(
                            w_sb[:, c, :],
                            w_e[:].rearrange("(c p) d -> p c d", p=128)[:, c, :],
                        )
                    for dc in range(NC):
                        ps = psA.tile([128, 1024], F32, tag="psA")
                        for qh in range(2):
                            o = qh * 512
                            for kc in range(NC):
                                nc.tensor.matmul(
                                    ps[:, o : o + 512],
                                    w_sb[:, kc, dc * 128 : dc * 128 + 128],
                                    xt_sb[:, kc, o : o + 512],
                                    start=(kc == 0),
                                    stop=(kc == NC - 1),
                                )
                        if dst is qt_sb:
                            nc.vector.tensor_scalar_add(
                                dst[:, dc, :], ps[:, :], b_sb[:, dc : dc + 1]
                            )
                        else:
                            nc.vector.tensor_scalar_add(
                                kt_sb[0:64, 2 * dc, :], ps[0:64, :],
                                b_sb[0:64, dc : dc + 1],
                            )
                            nc.vector.tensor_scalar_add(
                                kt_sb[64:128, 2 * dc + 1, :], ps[64:128, :],
                                b_sb[64:128, dc : dc + 1],
                            )

            # =========== attention + output projection ===========
            with (
                tc.tile_pool(name="w2", bufs=1) as w2,
                tc.tile_pool(name="et", bufs=6) as et_pool,
                tc.tile_pool(name="norm", bufs=1) as norm_pool,
            ):
                wo_sb = w2.tile([128, NC, H], CDT)
                for c in range(NC):
                    nc.sync.dma_start(
                        wo_sb[:, c, :],
                        wot_e[:].rearrange("(c p) d -> p c d", p=128)[:, c, :],
                    )

                def make_normalize(hp, ctxu_a, ctxu_b, ra, rb):
                    def emit():
                        # broadcast 1/denom over 64 partitions via f32r matmul,
                        # then scale ctx^T and store to ctxt_sb
                        for recip, ctxu, btag in ((ra, ctxu_a, "bca"),
                                                  (rb, ctxu_b, "bcb")):
                            bc = psA.tile([64, 1024], F32, tag="psA")
                            for o in (0, 512):
                                nc.tensor.matmul(
                                    bc[:, o : o + 512],
                                    ones[64:65, 0:64],
                                    recip[64:65, o : o + 512],
                                    start=True,
                                    stop=True,
                                )
                            bc_sb = norm_pool.tile([64, 1024], F32, tag=btag)
                            nc.vector.tensor_copy(bc_sb[:], bc[:])
                            if btag == "bca":
                                nc.gpsimd.tensor_tensor(
                                    ctxt_sb[0:64, hp, :], ctxu[0:64, :], bc_sb[:],
                                    mybir.AluOpType.mult,
                                )
                            else:
                                tmp_o = norm_pool.tile([64, 1024], CDT, tag="tmp")
                                nc.gpsimd.tensor_tensor(
                                    tmp_o[:], ctxu[0:64, :], bc_sb[:],
                                    mybir.AluOpType.mult,
                                )
                                # lift odd head to partitions 64:128 (DMA can
                                # cross partitions; DVE cannot)
                                nc.sync.dma_start(ctxt_sb[64:128, hp, :], tmp_o[:])
                    return emit

                pending = None
                for hp in range(NH // 2):
                    ha, hb = 2 * hp, 2 * hp + 1
                    # per head: rows 0:64 = ctx^T, row 64 = softmax denominator
                    ctx_a = psB.tile([128, 1024], F32, tag="psB")
                    ctx_b = psB.tile([128, 1024], F32, tag="psB")

                    def emit_pv(kc, et_a, et_b, ctx_a=ctx_a, ctx_b=ctx_b,
                                ha=ha, hb=hb):
                        first, last = kc == 0, kc == LC - 1
                        for qh in range(2):
                            o = qh * 512
                            # ctx^T[d, q] += V^T P^T ; row 64 = denominator
                            nc.tensor.matmul(
                                ctx_a[0 : HD + 1, o : o + 512],
                                v_sb[:, kc, ha, :],
                                et_a[:, o : o + 512],
                                start=first,
                                stop=last,
                            )
                            nc.tensor.matmul(
                                ctx_b[0 : HD + 1, o : o + 512],
                                v_sb[:, kc, hb, :],
                                et_b[:, o : o + 512],
                                start=first,
                                stop=last,
                            )

                    pv_q = []  # software pipeline: PV(kc-1) after ST(kc)
                    for kc in range(LC):
                        st_a = psA.tile([128, 1024], F32, tag="psA")
                        st_b = psA.tile([128, 1024], F32, tag="psA")
                        for qh in range(2):
                            o = qh * 512
                            # S^T[k, q] = K Q^T for both heads (row-group packed)
                            nc.tensor.matmul(
                                st_a[:, o : o + 512],
                                kt_sb[:, ha, kc * 128 : kc * 128 + 128],
                                qt_sb[:, hp, o : o + 512],
                                start=True,
                                stop=True,
                            )
                            nc.tensor.matmul(
                                st_b[:, o : o + 512],
                                kt_sb[:, hb, kc * 128 : kc * 128 + 128],
                                qt_sb[:, hp, o : o + 512],
                                start=True,
                                stop=True,
                            )
                        # P^T = exp(S^T/8 + mask_k)
                        et_a = et_pool.tile([128, 1024], CDT, tag="et")
                        et_b = et_pool.tile([128, 1024], CDT, tag="et")
                        nc.scalar.activation(
                            et_a[:], st_a[:], EXP,
                            bias=mask_sb[:, kc : kc + 1], scale=0.125,
                        )
                        nc.scalar.activation(
                            et_b[:], st_b[:], EXP,
                            bias=mask_sb[:, kc : kc + 1], scale=0.125,
                        )
                        pv_q.append((kc, et_a, et_b))
                        if kc >= 1:
                            emit_pv(*pv_q.pop(0))
                        if kc == 3 and pending is not None:
                            # previous pair's normalize, emitted here so its
                            # matmuls never head-of-line-block the PE
                            pending()
                            pending = None
                    emit_pv(*pv_q.pop(0))
                    # evacuate ctx+denominator to SBUF immediately: frees the
                    # PSUM slot and takes the reciprocal off the PE path
                    ctxu_a = norm_pool.tile([65, 1024], F32, tag="cua")
                    nc.vector.tensor_copy(ctxu_a[:], ctx_a[0:65, :])
                    ctxu_b = norm_pool.tile([65, 1024], F32, tag="cub")
                    nc.vector.tensor_copy(ctxu_b[:], ctx_b[0:65, :])
                    # 1/d = exp(-ln d) on ScalarE: Log+Exp share one ACT
                    # table set, ~1e-5 rel err, and it keeps the reciprocal
                    # off the (slow, 1-lane) DVE path entirely
                    LOG = mybir.ActivationFunctionType.Ln
                    lna = norm_pool.tile([65, 1024], F32, tag="lna")
                    nc.scalar.activation(lna[64:65, :], ctxu_a[64:65, :], LOG)
                    lnb = norm_pool.tile([65, 1024], F32, tag="lnb")
                    nc.scalar.activation(lnb[64:65, :], ctxu_b[64:65, :], LOG)
                    ra = norm_pool.tile([65, 1024], CDT, tag="ra")
                    rb = norm_pool.tile([65, 1024], CDT, tag="rb")
                    nc.scalar.activation(ra[64:65, :], lna[64:65, :], EXP, scale=-1.0)
                    nc.scalar.activation(rb[64:65, :], lnb[64:65, :], EXP, scale=-1.0)
                    pending = make_normalize(hp, ctxu_a, ctxu_b, ra, rb)
                if pending is not None:
                    pending()
                    pending = None

                # ---- output projection: out[q, o] = ctx Wo^T + bo ----
                for lc in range(LC):
                    ps = psA.tile([128, 1024], F32, tag="psA")
                    for off, width in ((0, 512), (512, 256)):
                        for c in range(NC):
                            nc.tensor.matmul(
                                ps[:, off : off + width],
                                ctxt_sb[:, c, lc * 128 : lc * 128 + 128],
                                wo_sb[:, c, off : off + width],
                                start=(c == 0),
                                stop=False,
                            )
                        nc.tensor.matmul(  # + bo
                            ps[:, off : off + width],
                            ones[0:1, 0:128],
                            bo_sb[0:1, off : off + width],
                            start=False,
                            stop=True,
                        )
                    o_sb = out_pool.tile([128, H], F32, tag="outp")
                    nc.vector.tensor_copy(o_sb[:], ps[:, 0:H])
                    nc.sync.dma_start(out_e[lc * 128 : lc * 128 + 128, :], o_sb[:])

    nc.finalize()
    nc.m = get_hw_module(nc.m)
    return nc


_NC_CACHE = {}


def _get_nc(compute_rounded: bool = True):
    if compute_rounded not in _NC_CACHE:
        _NC_CACHE[compute_rounded] = build_bass(compute_rounded)
    return _NC_CACHE[compute_rounded]


def make_in_maps(inputs):
    f = lambda a: np.ascontiguousarray(np.asarray(a, dtype=np.float32))  # noqa: E731
    hs = f(inputs["hidden_states"])
    mask = f(inputs["attention_mask"]).reshape(B, L)
    shared = {
        "wqt": f(np.asarray(inputs["Wq"]).T),
        "wkt": f(np.asarray(inputs["Wk"]).T),
        "wvt": f(np.asarray(inputs["Wv"]).T),
        "wot": f(np.asarray(inputs["Wo"]).T),
        "bq": f(inputs["bq"]),
        "bk": f(inputs["bk"]),
        "bv": f(inputs["bv"]),
        "bo": f(inputs["bo"]),
    }
    return [
        {"xt": f(hs[b].T), "mask": mask[b], **shared}
        for b in range(B)
    ]


def run_spmd(inputs, trace=False, compute_rounded=True):
    nc = _get_nc(compute_rounded)
    res = run_bass_kernel_spmd(nc, make_in_maps(inputs), list(range(B)), trace=trace)
    out = np.stack([res.results[b]["out"] for b in range(B)]).astype(np.float32)
    return out, res


def kernel(**inputs) -> np.ndarray:
    out, _ = run_spmd(inputs, trace=False)
    return out


# revision 14
# speedup vs baseline: 1.4553x; 1.0067x over previous
"""BERT self-attention on 8 Trainium2 NeuronCores.

Sharding: data-parallel over batch (B=8 -> one batch element per core).
Each core computes full self-attention for its batch element:
  Q/K/V projections, per-head softmax(Q K^T / 8 + mask) V, output proj.

Layout strategy (per core):
  - Host passes xT = x.T [768,1024] and W.T [768,768] so every matmul
    contracts over the partition axis.
  - QT,KT [d, L] and V [L, d] are produced directly by the projections.
  - Attention runs transposed: ST[k,q] = K Q^T per head, so softmax's
    reduction axis (k) lands on partitions: exp via ScalarE with the
    attention mask as per-partition bias (no max subtraction: scores are
    ~N(0,1), |s|<~7, exp is safe in fp32); the denominator comes from a
    ones column appended to V (out row 64); P^T V accumulates ctx^T
    [d, q] which feeds the output projection as lhsT directly.
  - Matmul inputs are float32r (full PE speed at N>=512); Q^T/K^T are
    bf16 with K^T zero-padded per head to a full 128-row contraction
    (K=64 matmuls stream at half rate on TRN2). 1/denom is computed as
    exp(-ln d) on ScalarE (same ACT table set as the softmax exp).
    Accumulation and softmax run in fp32.
"""

import numpy as np

import concourse.bass as bass  # noqa: F401
import concourse.mybir as mybir
import concourse.tile as tile
from concourse import bacc
from concourse.bass_interp import get_hw_module
from concourse.bass_utils import run_bass_kernel_spmd

B, L, H = 8, 1024, 768
NH, HD = 12, 64
NC = H // 128          # 6 chunks of hidden dim
LC = L // 128          # 8 chunks of sequence dim
F32 = mybir.dt.float32
EXP = mybir.ActivationFunctionType.Exp


def build_bass(compute_rounded: bool = True):
    CDT = mybir.dt.float32r if compute_rounded else F32

    nc = bacc.Bacc("TRN2", debug=False, num_devices=8)

    # The kernel uses Exp (softmax) and Ln (reciprocal via exp(-ln d)).
    # Walrus's table-load pass would alternate exp_and_others /
    # natural_log sets (a ~1.3us ACT table DMA per switch, dozens per
    # kernel). Make the combined natural_log_exp_and_others set the only
    # provider of Exp/Ln so a single table load serves the whole kernel.
    from concourse.hw_specs import get_activation_tables

    _tabs = get_activation_tables(nc.m.arch)
    _E = mybir.ActivationFunctionType.Exp
    _L = mybir.ActivationFunctionType.Ln
    if "natural_log_exp_and_others" in _tabs:
        for _name, _fns in _tabs.items():
            if _name != "natural_log_exp_and_others":
                _fns.discard(_E)
                _fns.discard(_L)

    xt_e = nc.declare_dram_parameter("xt", [H, L], CDT, isOutput=False)
    wqt_e = nc.declare_dram_parameter("wqt", [H, H], CDT, isOutput=False)
    wkt_e = nc.declare_dram_parameter("wkt", [H, H], CDT, isOutput=False)
    wvt_e = nc.declare_dram_parameter("wvt", [H, H], CDT, isOutput=False)
    wot_e = nc.declare_dram_parameter("wot", [H, H], CDT, isOutput=False)
    bq_e = nc.declare_dram_parameter("bq", [H], F32, isOutput=False)
    bk_e = nc.declare_dram_parameter("bk", [H], F32, isOutput=False)
    bv_e = nc.declare_dram_parameter("bv", [H], CDT, isOutput=False)
    bo_e = nc.declare_dram_parameter("bo", [H], CDT, isOutput=False)
    mask_e = nc.declare_dram_parameter("mask", [L], F32, isOutput=False)
    out_e = nc.declare_dram_parameter("out", [L, H], F32, isOutput=True)

    with tile.TileContext(nc) as tc:
        with (
            tc.tile_pool(name="small", bufs=1) as small,
            tc.tile_pool(name="acts", bufs=1) as acts,
            tc.tile_pool(name="outp", bufs=2) as out_pool,
            tc.tile_pool(name="psA", bufs=2, space="PSUM") as psA,
            tc.tile_pool(name="psB", bufs=2, space="PSUM") as psB,
        ):
            # ---- constants / small tensors ----
            mask_sb = small.tile([128, LC], F32)
            nc.sync.dma_start(mask_sb[:], mask_e[:].rearrange("(c p) -> p c", p=128))
            bq_sb = small.tile([128, NC], F32)
            nc.sync.dma_start(bq_sb[:], bq_e[:].rearrange("(c p) -> p c", p=128))
            bk_sb = small.tile([128, NC], F32)
            nc.sync.dma_start(bk_sb[:], bk_e[:].rearrange("(c p) -> p c", p=128))
            bv_sb = small.tile([1, H], CDT)
            nc.sync.dma_start(bv_sb[:], bv_e[None, :])
            bo_sb = small.tile([1, H], CDT)
            nc.sync.dma_start(bo_sb[:], bo_e[None, :])
            ones32 = small.tile([128, 128], F32)
            nc.vector.memset(ones32[:], 1.0)
            ones = small.tile([128, 128], CDT)
            nc.vector.tensor_copy(ones[:], ones32[:])

            BF = mybir.dt.bfloat16
            qt_sb = acts.tile([128, NC, L], BF)
            kt_sb = acts.tile([128, NH, L], BF)  # per-head K^T, other 64 rows zero
            nc.gpsimd.memset(kt_sb[:], 0.0)
            v_sb = acts.tile([128, LC, NH, HD + 1], CDT)  # [..., 64] = ones col
            ctxt_sb = acts.tile([128, NC, L], CDT)

            nc.vector.tensor_copy(
                v_sb[:, :, :, HD],
                ones32[:, 0 : LC * NH].rearrange("p (a b) -> p a b", a=LC),
            )

            # =========== projection phase (xt + wv/wk/wq scoped) ===========
            with (
                tc.tile_pool(name="xt", bufs=1) as xt_pool,
                tc.tile_pool(name="w1", bufs=2) as w1,
            ):
                xt_sb = xt_pool.tile([128, NC, L], CDT)
                for c in range(NC):
                    nc.sync.dma_start(
                        xt_sb[:, c, :],
                        xt_e[:].rearrange("(c p) q -> p c q", p=128)[:, c, :],
                    )

                # ---- V projection: V[l, d] = x Wv^T + bv (natural layout)
                wv_sb = w1.tile([128, NC, H], CDT, tag="w")
                for c in range(NC):
                    nc.sync.dma_start(
                        wv_sb[:, c, :],
                        wvt_e[:].rearrange("(c p) d -> p c d", p=128)[:, c, :],
                    )
                for lc in range(LC):
                    ps = psA.tile([128, 1024], F32, tag="psA")
                    for off, width in ((0, 512), (512, 256)):
                        for kc in range(NC):
                            nc.tensor.matmul(
                                ps[:, off : off + width],
                                xt_sb[:, kc, lc * 128 : lc * 128 + 128],
                                wv_sb[:, kc, off : off + width],
                                start=(kc == 0),
                                stop=False,
                            )
                        nc.tensor.matmul(  # + bv (ones row x bias row)
                            ps[:, off : off + width],
                            ones[0:1, 0:128],
                            bv_sb[0:1, off : off + width],
                            start=False,
                            stop=True,
                        )
                    nc.vector.tensor_copy(
                        v_sb[:, lc, :, 0:HD],
                        ps[:, 0:H].rearrange("p (h d) -> p h d", d=HD),
                    )

                # ---- K^T then Q^T projections: out[d, q] = W x^T + b
                for w_e, b_sb, dst in ((wkt_e, bk_sb, kt_sb), (wqt_e, bq_sb, qt_sb)):
                    w_sb = w1.tile([128, NC, H], CDT, tag="w")
                    for c in range(NC):
                        nc.sync.dma_start(
                            w_sb[:, c, :],
                            w_e[:].rearrange("(c p) d -> p c d", p=128)[:, c, :],
                        )
                    for dc in range(NC):
                        ps = psA.tile([128, 1024], F32, tag="psA")
                        for qh in range(2):
                            o = qh * 512
                            for kc in range(NC):
                                nc.tensor.matmul(
                                    ps[:, o : o + 512],
                                    w_sb[:, kc, dc * 128 : dc * 128 + 128],
                                    xt_sb[:, kc, o : o + 512],
                                    start=(kc == 0),
                                    stop=(kc == NC - 1),
                                )
                        if dst is qt_sb:
                            nc.vector.tensor_scalar_add(
                                dst[:, dc, :], ps[:, :], b_sb[:, dc : dc + 1]
                            )
                        else:
                            nc.vector.tensor_scalar_add(
                                kt_sb[0:64, 2 * dc, :], ps[0:64, :],
                                b_sb[0:64, dc : dc + 1],
                            )
                            nc.vector.tensor_scalar_add(
                                kt_sb[64:128, 2 * dc + 1, :], ps[64:128, :],
                                b_sb[64:128, dc : dc + 1],
                            )

            # =========== attention + output projection ===========
            with (
                tc.tile_pool(name="w2", bufs=1) as w2,
                tc.tile_pool(name="et", bufs=6) as et_pool,
                tc.tile_pool(name="norm", bufs=1) as norm_pool,
            ):
                wo_sb = w2.tile([128, NC, H], CDT)
                for c in range(NC):
                    nc.sync.dma_start(
                        wo_sb[:, c, :],
                        wot_e[:].rearrange("(c p) d -> p c d", p=128)[:, c, :],
                    )

                def make_normalize(hp, ctxu_a, ctxu_b, ra, rb):
                    def emit():
                        # broadcast 1/denom over 64 partitions via f32r matmul,
                        # then scale ctx^T and store to ctxt_sb
                        for recip, ctxu, btag in ((ra, ctxu_a, "bca"),
                                                  (rb, ctxu_b, "bcb")):
                            bc = psA.tile([64, 1024], F32, tag="psA")
                            for o in (0, 512):
                                nc.tensor.matmul(
                                    bc[:, o : o + 512],
                                    ones[64:65, 0:64],
                                    recip[64:65, o : o + 512],
                                    start=True,
                                    stop=True,
                                )
                            bc_sb = norm_pool.tile([64, 1024], F32, tag=btag)
                            nc.vector.tensor_copy(bc_sb[:], bc[:])
                            if btag == "bca":
                                nc.gpsimd.tensor_tensor(
                                    ctxt_sb[0:64, hp, :], ctxu[0:64, :], bc_sb[:],
                                    mybir.AluOpType.mult,
                                )
                            else:
                                tmp_o = norm_pool.tile([64, 1024], CDT, tag="tmp")
                                nc.gpsimd.tensor_tensor(
                                    tmp_o[:], ctxu[0:64, :], bc_sb[:],
                                    mybir.AluOpType.mult,
                                )
                                # lift odd head to partitions 64:128 (DMA can
                                # cross partitions; DVE cannot)
                                nc.sync.dma_start(ctxt_sb[64:128, hp, :], tmp_o[:])
                    return emit

                pending = None
                for hp in range(NH // 2):
                    ha, hb = 2 * hp, 2 * hp + 1
                    # per head: rows 0:64 = ctx^T, row 64 = softmax denominator
                    ctx_a = psB.tile([128, 1024], F32, tag="psB")
                    ctx_b = psB.tile([128, 1024], F32, tag="psB")

                    def emit_pv(kc, et_a, et_b, ctx_a=ctx_a, ctx_b=ctx_b,
                                ha=ha, hb=hb):
                        first, last = kc == 0, kc == LC - 1
                        for qh in range(2):
                            o = qh * 512
                            # ctx^T[d, q] += V^T P^T ; row 64 = denominator
                            nc.tensor.matmul(
                                ctx_a[0 : HD + 1, o : o + 512],
                                v_sb[:, kc, ha, :],
                                et_a[:, o : o + 512],
                                start=first,
                                stop=last,
                            )
                            nc.tensor.matmul(
                                ctx_b[0 : HD + 1, o : o + 512],
                                v_sb[:, kc, hb, :],
                                et_b[:, o : o + 512],
                                start=first,
                                stop=last,
                            )

                    pv_q = []  # software pipeline: PV(kc-1) after ST(kc)
                    for kc in range(LC):
                        st_a = psA.tile([128, 1024], F32, tag="psA")
                        st_b = psA.tile([128, 1024], F32, tag="psA")
                        for qh in range(2):
                            o = qh * 512
                            # S^T[k, q] = K Q^T for both heads (row-group packed)
                            nc.tensor.matmul(
                                st_a[:, o : o + 512],
                                kt_sb[:, ha, kc * 128 : kc * 128 + 128],
                                qt_sb[:, hp, o : o + 512],
                                start=True,
                                stop=True,
                            )
                            nc.tensor.matmul(
                                st_b[:, o : o + 512],
                                kt_sb[:, hb, kc * 128 : kc * 128 + 128],
                                qt_sb[:, hp, o : o + 512],
                                start=True,
                                stop=True,
                            )
                        # P^T = exp(S^T/8 + mask_k)
                        et_a = et_pool.tile([128, 1024], CDT, tag="et")
                        et_b = et_pool.tile([128, 1024], CDT, tag="et")
                        nc.scalar.activation(
                            et_a[:], st_a[:], EXP,
                            bias=mask_sb[:, kc : kc + 1], scale=0.125,
                        )
                        nc.scalar.activation(
                            et_b[:], st_b[:], EXP,
                            bias=mask_sb[:, kc : kc + 1], scale=0.125,
                        )
                        pv_q.append((kc, et_a, et_b))
                        if kc >= 1:
                            emit_pv(*pv_q.pop(0))
                        if kc == 3 and pending is not None:
                            # previous pair's normalize, emitted here so its
                            # matmuls never head-of-line-block the PE
                            pending()
                            pending = None
                    emit_pv(*pv_q.pop(0))
                    # evacuate ctx+denominator to SBUF immediately: frees the
                    # PSUM slot and takes the reciprocal off the PE path
                    ctxu_a = norm_pool.tile([65, 1024], F32, tag="cua")
                    nc.vector.tensor_copy(ctxu_a[:], ctx_a[0:65, :])
                    ctxu_b = norm_pool.tile([65, 1024], F32, tag="cub")
                    nc.vector.tensor_copy(ctxu_b[:], ctx_b[0:65, :])
                    # 1/d = exp(-ln d) on ScalarE: Log+Exp share one ACT
                    # table set, ~1e-5 rel err, and it keeps the reciprocal
                    # off the (slow, 1-lane) DVE path entirely
                    LOG = mybir.ActivationFunctionType.Ln
                    lna = norm_pool.tile([65, 1024], F32, tag="lna")
                    nc.scalar.activation(lna[64:65, :], ctxu_a[64:65, :], LOG)
                    lnb = norm_pool.tile([65, 1024], F32, tag="lnb")
                    nc.scalar.activation(lnb[64:65, :], ctxu_b[64:65, :], LOG)
                    ra = norm_pool.tile([65, 1024], CDT, tag="ra")
                    rb = norm_pool.tile([65, 1024], CDT, tag="rb")
                    nc.scalar.activation(ra[64:65, :], lna[64:65, :], EXP, scale=-1.0)
                    nc.scalar.activation(rb[64:65, :], lnb[64:65, :], EXP, scale=-1.0)
                    pending = make_normalize(hp, ctxu_a, ctxu_b, ra, rb)
                if pending is not None:
                    pending()
                    pending = None

                # ---- output projection: out[q, o] = ctx Wo^T + bo ----
                for lc in range(LC):
                    ps = psA.tile([128, 1024], F32, tag="psA")
                    for off, width in ((0, 512), (512, 256)):
                        for c in range(NC):
                            nc.tensor.matmul(
                                ps[:, off : off + width],
                                ctxt_sb[:, c, lc * 128 : lc * 128 + 128],
                                wo_sb[:, c, off : off + width],
                                start=(c == 0),
                                stop=False,
                            )
                        nc.tensor.matmul(  # + bo
                            ps[:, off : off + width],
                            ones[0:1, 0:128],
                            bo_sb[0:1, off : off + width],
                            start=False,
                            stop=True,
                        )
                    o_sb = out_pool.tile([128, H], F32, tag="outp")
                    nc.vector.tensor_copy(o_sb[:], ps[:, 0:H])
                    nc.sync.dma_start(out_e[lc * 128 : lc * 128 + 128, :], o_sb[:])

    nc.finalize()
    nc.m = get_hw_module(nc.m)
    return nc


_NC_CACHE = {}


def _get_nc(compute_rounded: bool = True):
    if compute_rounded not in _NC_CACHE:
        _NC_CACHE[compute_rounded] = build_bass(compute_rounded)
    return _NC_CACHE[compute_rounded]


def make_in_maps(inputs):
    f = lambda a: np.ascontiguousarray(np.asarray(a, dtype=np.float32))  # noqa: E731
    hs = f(inputs["hidden_states"])
    mask = f(inputs["attention_mask"]).reshape(B, L)
    shared = {
        "wqt": f(np.asarray(inputs["Wq"]).T),
        "wkt": f(np.asarray(inputs["Wk"]).T),
        "wvt": f(np.asarray(inputs["Wv"]).T),
        "wot": f(np.asarray(inputs["Wo"]).T),
        "bq": f(inputs["bq"]),
        "bk": f(inputs["bk"]),
        "bv": f(inputs["bv"]),
        "bo": f(inputs["bo"]),
    }
    return [
        {"xt": f(hs[b].T), "mask": mask[b], **shared}
        for b in range(B)
    ]


def run_spmd(inputs, trace=False, compute_rounded=True):
    nc = _get_nc(compute_rounded)
    res = run_bass_kernel_spmd(nc, make_in_maps(inputs), list(range(B)), trace=trace)
    out = np.stack([res.results[b]["out"] for b in range(B)]).astype(np.float32)
    return out, res


def kernel(**inputs) -> np.ndarray:
    out, _ = run_spmd(inputs, trace=False)
    return out


# revision 15
# speedup vs baseline: 1.4677x; 1.0085x over previous
"""BERT self-attention on 8 Trainium2 NeuronCores.

Sharding: data-parallel over batch (B=8 -> one batch element per core).
Each core computes full self-attention for its batch element:
  Q/K/V projections, per-head softmax(Q K^T / 8 + mask) V, output proj.

Layout strategy (per core):
  - Host passes xT = x.T [768,1024] and W.T [768,768] so every matmul
    contracts over the partition axis.
  - QT,KT [d, L] and V [L, d] are produced directly by the projections.
  - Attention runs transposed: ST[k,q] = K Q^T per head, so softmax's
    reduction axis (k) lands on partitions: exp via ScalarE with the
    attention mask as per-partition bias (no max subtraction: scores are
    ~N(0,1), |s|<~7, exp is safe in fp32); the denominator comes from a
    ones column appended to V (out row 64); P^T V accumulates ctx^T
    [d, q] which feeds the output projection as lhsT directly.
  - Matmul inputs are float32r (full PE speed at N>=512); Q^T/K^T are
    bf16 with K^T zero-padded per head to a full 128-row contraction
    (K=64 matmuls stream at half rate on TRN2). 1/denom is computed as
    exp(-ln d) on ScalarE (same ACT table set as the softmax exp).
    Accumulation and softmax run in fp32.
"""

import numpy as np

import concourse.bass as bass  # noqa: F401
import concourse.mybir as mybir
import concourse.tile as tile
from concourse import bacc
from concourse.bass_interp import get_hw_module
from concourse.bass_utils import run_bass_kernel_spmd

B, L, H = 8, 1024, 768
NH, HD = 12, 64
NC = H // 128          # 6 chunks of hidden dim
LC = L // 128          # 8 chunks of sequence dim
F32 = mybir.dt.float32
EXP = mybir.ActivationFunctionType.Exp


def build_bass(compute_rounded: bool = True):
    CDT = mybir.dt.float32r if compute_rounded else F32

    nc = bacc.Bacc("TRN2", debug=False, num_devices=8)

    # The kernel uses Exp (softmax) and Ln (reciprocal via exp(-ln d)).
    # Walrus's table-load pass would alternate exp_and_others /
    # natural_log sets (a ~1.3us ACT table DMA per switch, dozens per
    # kernel). Make the combined natural_log_exp_and_others set the only
    # provider of Exp/Ln so a single table load serves the whole kernel.
    from concourse.hw_specs import get_activation_tables

    _tabs = get_activation_tables(nc.m.arch)
    _E = mybir.ActivationFunctionType.Exp
    _L = mybir.ActivationFunctionType.Ln
    if "natural_log_exp_and_others" in _tabs:
        for _name, _fns in _tabs.items():
            if _name != "natural_log_exp_and_others":
                _fns.discard(_E)
                _fns.discard(_L)

    xt_e = nc.declare_dram_parameter("xt", [H, L], CDT, isOutput=False)
    wqt_e = nc.declare_dram_parameter("wqt", [H, H], CDT, isOutput=False)
    wkt_e = nc.declare_dram_parameter("wkt", [H, H], CDT, isOutput=False)
    wvt_e = nc.declare_dram_parameter("wvt", [H, H], CDT, isOutput=False)
    wot_e = nc.declare_dram_parameter("wot", [H, H], CDT, isOutput=False)
    bq_e = nc.declare_dram_parameter("bq", [H], F32, isOutput=False)
    bk_e = nc.declare_dram_parameter("bk", [H], F32, isOutput=False)
    bv_e = nc.declare_dram_parameter("bv", [H], CDT, isOutput=False)
    bo_e = nc.declare_dram_parameter("bo", [H], CDT, isOutput=False)
    mask_e = nc.declare_dram_parameter("mask", [L], F32, isOutput=False)
    out_e = nc.declare_dram_parameter("out", [L, H], F32, isOutput=True)

    with tile.TileContext(nc) as tc:
        with (
            tc.tile_pool(name="small", bufs=1) as small,
            tc.tile_pool(name="acts", bufs=1) as acts,
            tc.tile_pool(name="outp", bufs=2) as out_pool,
            tc.tile_pool(name="psA", bufs=2, space="PSUM") as psA,
            tc.tile_pool(name="psB", bufs=2, space="PSUM") as psB,
        ):
            # ---- constants / small tensors ----
            mask_sb = small.tile([128, LC], F32)
            nc.sync.dma_start(mask_sb[:], mask_e[:].rearrange("(c p) -> p c", p=128))
            bq_sb = small.tile([128, NC], F32)
            nc.sync.dma_start(bq_sb[:], bq_e[:].rearrange("(c p) -> p c", p=128))
            bk_sb = small.tile([128, NC], F32)
            nc.sync.dma_start(bk_sb[:], bk_e[:].rearrange("(c p) -> p c", p=128))
            bv_sb = small.tile([1, H], CDT)
            nc.sync.dma_start(bv_sb[:], bv_e[None, :])
            bo_sb = small.tile([1, H], CDT)
            nc.sync.dma_start(bo_sb[:], bo_e[None, :])
            ones32 = small.tile([128, 128], F32)
            nc.vector.memset(ones32[:], 1.0)
            ones = small.tile([128, 128], CDT)
            nc.vector.tensor_copy(ones[:], ones32[:])

            BF = mybir.dt.bfloat16
            qt_sb = acts.tile([128, NC, L], BF)
            kt_sb = acts.tile([128, NH, L], BF)  # per-head K^T, other 64 rows zero
            nc.gpsimd.memset(kt_sb[:], 0.0)
            v_sb = acts.tile([128, LC, NH, HD + 1], CDT)  # [..., 64] = ones col
            ctxt_sb = acts.tile([128, NC, L], CDT)

            nc.vector.tensor_copy(
                v_sb[:, :, :, HD],
                ones32[:, 0 : LC * NH].rearrange("p (a b) -> p a b", a=LC),
            )

            # =========== projection phase (xt + wv/wk/wq scoped) ===========
            with (
                tc.tile_pool(name="xt", bufs=1) as xt_pool,
                tc.tile_pool(name="w1", bufs=2) as w1,
            ):
                # interleave xt/wv chunk DMAs: V-proj's first matmuls only
                # need chunk 0 of each, so compute starts ~3us in instead of
                # waiting for the full 5.25MB load
                xt_sb = xt_pool.tile([128, NC, L], CDT)
                wv_sb = w1.tile([128, NC, H], CDT, tag="w")
                for c in range(NC):
                    nc.sync.dma_start(
                        wv_sb[:, c, :],
                        wvt_e[:].rearrange("(c p) d -> p c d", p=128)[:, c, :],
                    )
                    nc.sync.dma_start(
                        xt_sb[:, c, :],
                        xt_e[:].rearrange("(c p) q -> p c q", p=128)[:, c, :],
                    )
                for lc in range(LC):
                    ps = psA.tile([128, 1024], F32, tag="psA")
                    for off, width in ((0, 512), (512, 256)):
                        for kc in range(NC):
                            nc.tensor.matmul(
                                ps[:, off : off + width],
                                xt_sb[:, kc, lc * 128 : lc * 128 + 128],
                                wv_sb[:, kc, off : off + width],
                                start=(kc == 0),
                                stop=False,
                            )
                        nc.tensor.matmul(  # + bv (ones row x bias row)
                            ps[:, off : off + width],
                            ones[0:1, 0:128],
                            bv_sb[0:1, off : off + width],
                            start=False,
                            stop=True,
                        )
                    nc.vector.tensor_copy(
                        v_sb[:, lc, :, 0:HD],
                        ps[:, 0:H].rearrange("p (h d) -> p h d", d=HD),
                    )

                # ---- K^T then Q^T projections: out[d, q] = W x^T + b
                for w_e, b_sb, dst in ((wkt_e, bk_sb, kt_sb), (wqt_e, bq_sb, qt_sb)):
                    w_sb = w1.tile([128, NC, H], CDT, tag="w")
                    for c in range(NC):
                        nc.sync.dma_start(
                            w_sb[:, c, :],
                            w_e[:].rearrange("(c p) d -> p c d", p=128)[:, c, :],
                        )
                    for dc in range(NC):
                        ps = psA.tile([128, 1024], F32, tag="psA")
                        for qh in range(2):
                            o = qh * 512
                            for kc in range(NC):
                                nc.tensor.matmul(
                                    ps[:, o : o + 512],
                                    w_sb[:, kc, dc * 128 : dc * 128 + 128],
                                    xt_sb[:, kc, o : o + 512],
                                    start=(kc == 0),
                                    stop=(kc == NC - 1),
                                )
                        if dst is qt_sb:
                            nc.vector.tensor_scalar_add(
                                dst[:, dc, :], ps[:, :], b_sb[:, dc : dc + 1]
                            )
                        else:
                            nc.vector.tensor_scalar_add(
                                kt_sb[0:64, 2 * dc, :], ps[0:64, :],
                                b_sb[0:64, dc : dc + 1],
                            )
                            nc.vector.tensor_scalar_add(
                                kt_sb[64:128, 2 * dc + 1, :], ps[64:128, :],
                                b_sb[64:128, dc : dc + 1],
                            )

            # =========== attention + output projection ===========
            with (
                tc.tile_pool(name="w2", bufs=1) as w2,
                tc.tile_pool(name="et", bufs=6) as et_pool,
                tc.tile_pool(name="norm", bufs=1) as norm_pool,
            ):
                wo_sb = w2.tile([128, NC, H], CDT)
                for c in range(NC):
                    nc.sync.dma_start(
                        wo_sb[:, c, :],
                        wot_e[:].rearrange("(c p) d -> p c d", p=128)[:, c, :],
                    )

                def make_normalize(hp, ctxu_a, ctxu_b, ra, rb):
                    def emit():
                        # broadcast 1/denom over 64 partitions via f32r matmul,
                        # then scale ctx^T and store to ctxt_sb
                        for recip, ctxu, btag in ((ra, ctxu_a, "bca"),
                                                  (rb, ctxu_b, "bcb")):
                            bc = psA.tile([64, 1024], F32, tag="psA")
                            for o in (0, 512):
                                nc.tensor.matmul(
                                    bc[:, o : o + 512],
                                    ones[64:65, 0:64],
                                    recip[64:65, o : o + 512],
                                    start=True,
                                    stop=True,
                                )
                            bc_sb = norm_pool.tile([64, 1024], F32, tag=btag)
                            nc.vector.tensor_copy(bc_sb[:], bc[:])
                            if btag == "bca":
                                nc.gpsimd.tensor_tensor(
                                    ctxt_sb[0:64, hp, :], ctxu[0:64, :], bc_sb[:],
                                    mybir.AluOpType.mult,
                                )
                            else:
                                tmp_o = norm_pool.tile([64, 1024], CDT, tag="tmp")
                                nc.gpsimd.tensor_tensor(
                                    tmp_o[:], ctxu[0:64, :], bc_sb[:],
                                    mybir.AluOpType.mult,
                                )
                                # lift odd head to partitions 64:128 (DMA can
                                # cross partitions; DVE cannot)
                                nc.sync.dma_start(ctxt_sb[64:128, hp, :], tmp_o[:])
                    return emit

                pending = None
                for hp in range(NH // 2):
                    ha, hb = 2 * hp, 2 * hp + 1
                    # per head: rows 0:64 = ctx^T, row 64 = softmax denominator
                    ctx_a = psB.tile([128, 1024], F32, tag="psB")
                    ctx_b = psB.tile([128, 1024], F32, tag="psB")

                    def emit_pv(kc, et_a, et_b, ctx_a=ctx_a, ctx_b=ctx_b,
                                ha=ha, hb=hb):
                        first, last = kc == 0, kc == LC - 1
                        for qh in range(2):
                            o = qh * 512
                            # ctx^T[d, q] += V^T P^T ; row 64 = denominator
                            nc.tensor.matmul(
                                ctx_a[0 : HD + 1, o : o + 512],
                                v_sb[:, kc, ha, :],
                                et_a[:, o : o + 512],
                                start=first,
                                stop=last,
                            )
                            nc.tensor.matmul(
                                ctx_b[0 : HD + 1, o : o + 512],
                                v_sb[:, kc, hb, :],
                                et_b[:, o : o + 512],
                                start=first,
                                stop=last,
                            )

                    pv_q = []  # software pipeline: PV(kc-1) after ST(kc)
                    for kc in range(LC):
                        st_a = psA.tile([128, 1024], F32, tag="psA")
                        st_b = psA.tile([128, 1024], F32, tag="psA")
                        for qh in range(2):
                            o = qh * 512
                            # S^T[k, q] = K Q^T for both heads (row-group packed)
                            nc.tensor.matmul(
                                st_a[:, o : o + 512],
                                kt_sb[:, ha, kc * 128 : kc * 128 + 128],
                                qt_sb[:, hp, o : o + 512],
                                start=True,
                                stop=True,
                            )
                            nc.tensor.matmul(
                                st_b[:, o : o + 512],
                                kt_sb[:, hb, kc * 128 : kc * 128 + 128],
                                qt_sb[:, hp, o : o + 512],
                                start=True,
                                stop=True,
                            )
                        # P^T = exp(S^T/8 + mask_k)
                        et_a = et_pool.tile([128, 1024], CDT, tag="et")
                        et_b = et_pool.tile([128, 1024], CDT, tag="et")
                        nc.scalar.activation(
                            et_a[:], st_a[:], EXP,
                            bias=mask_sb[:, kc : kc + 1], scale=0.125,
                        )
                        nc.scalar.activation(
                            et_b[:], st_b[:], EXP,
                            bias=mask_sb[:, kc : kc + 1], scale=0.125,
                        )
                        pv_q.append((kc, et_a, et_b))
                        if kc >= 1:
                            emit_pv(*pv_q.pop(0))
                        if kc == 3 and pending is not None:
                            # previous pair's normalize, emitted here so its
                            # matmuls never head-of-line-block the PE
                            pending()
                            pending = None
                    emit_pv(*pv_q.pop(0))
                    # evacuate ctx+denominator to SBUF immediately: frees the
                    # PSUM slot and takes the reciprocal off the PE path
                    ctxu_a = norm_pool.tile([65, 1024], F32, tag="cua")
                    nc.vector.tensor_copy(ctxu_a[:], ctx_a[0:65, :])
                    ctxu_b = norm_pool.tile([65, 1024], F32, tag="cub")
                    nc.vector.tensor_copy(ctxu_b[:], ctx_b[0:65, :])
                    # 1/d = exp(-ln d) on ScalarE: Log+Exp share one ACT
                    # table set, ~1e-5 rel err, and it keeps the reciprocal
                    # off the (slow, 1-lane) DVE path entirely
                    LOG = mybir.ActivationFunctionType.Ln
                    ra32 = norm_pool.tile([65, 1024], F32, tag="ra32")
                    nc.vector.reciprocal(ra32[64:65, :], ctxu_a[64:65, :])
                    ra = norm_pool.tile([65, 1024], CDT, tag="ra")
                    nc.vector.tensor_copy(ra[64:65, :], ra32[64:65, :])
                    lnb = norm_pool.tile([65, 1024], F32, tag="lnb")
                    nc.scalar.activation(lnb[64:65, :], ctxu_b[64:65, :], LOG)
                    rb = norm_pool.tile([65, 1024], CDT, tag="rb")
                    nc.scalar.activation(rb[64:65, :], lnb[64:65, :], EXP, scale=-1.0)
                    pending = make_normalize(hp, ctxu_a, ctxu_b, ra, rb)
                if pending is not None:
                    pending()
                    pending = None

                # ---- output projection: out[q, o] = ctx Wo^T + bo ----
                for lc in range(LC):
                    ps = psA.tile([128, 1024], F32, tag="psA")
                    for off, width in ((0, 512), (512, 256)):
                        for c in range(NC):
                            nc.tensor.matmul(
                                ps[:, off : off + width],
                                ctxt_sb[:, c, lc * 128 : lc * 128 + 128],
                                wo_sb[:, c, off : off + width],
                                start=(c == 0),
                                stop=False,
                            )
                        nc.tensor.matmul(  # + bo
                            ps[:, off : off + width],
                            ones[0:1, 0:128],
                            bo_sb[0:1, off : off + width],
                            start=False,
                            stop=True,
                        )
                    o_sb = out_pool.tile([128, H], F32, tag="outp")
                    nc.vector.tensor_copy(o_sb[:], ps[:, 0:H])
                    nc.sync.dma_start(out_e[lc * 128 : lc * 128 + 128, :], o_sb[:])

    nc.finalize()
    nc.m = get_hw_module(nc.m)
    return nc


_NC_CACHE = {}


def _get_nc(compute_rounded: bool = True):
    if compute_rounded not in _NC_CACHE:
        _NC_CACHE[compute_rounded] = build_bass(compute_rounded)
    return _NC_CACHE[compute_rounded]


def make_in_maps(inputs):
    f = lambda a: np.ascontiguousarray(np.asarray(a, dtype=np.float32))  # noqa: E731
    hs = f(inputs["hidden_states"])
    mask = f(inputs["attention_mask"]).reshape(B, L)
    shared = {
        "wqt": f(np.asarray(inputs["Wq"]).T),
        "wkt": f(np.asarray(inputs["Wk"]).T),
        "wvt": f(np.asarray(inputs["Wv"]).T),
        "wot": f(np.asarray(inputs["Wo"]).T),
        "bq": f(inputs["bq"]),
        "bk": f(inputs["bk"]),
        "bv": f(inputs["bv"]),
        "bo": f(inputs["bo"]),
    }
    return [
        {"xt": f(hs[b].T), "mask": mask[b], **shared}
        for b in range(B)
    ]


def run_spmd(inputs, trace=False, compute_rounded=True):
    nc = _get_nc(compute_rounded)
    res = run_bass_kernel_spmd(nc, make_in_maps(inputs), list(range(B)), trace=trace)
    out = np.stack([res.results[b]["out"] for b in range(B)]).astype(np.float32)
    return out, res


def kernel(**inputs) -> np.ndarray:
    out, _ = run_spmd(inputs, trace=False)
    return out


# revision 16
# speedup vs baseline: 1.5103x; 1.0290x over previous
"""BERT self-attention on 8 Trainium2 NeuronCores.

Sharding: data-parallel over batch (B=8 -> one batch element per core).
Each core computes full self-attention for its batch element:
  Q/K/V projections, per-head softmax(Q K^T / 8 + mask) V, output proj.

Layout strategy (per core):
  - Host passes xT = x.T [768,1024] and W.T [768,768] so every matmul
    contracts over the partition axis.
  - QT,KT [d, L] and V [L, d] are produced directly by the projections.
  - Attention runs transposed: ST[k,q] = K Q^T per head, so softmax's
    reduction axis (k) lands on partitions: exp via ScalarE with the
    attention mask as per-partition bias (no max subtraction: scores are
    ~N(0,1), |s|<~7, exp is safe in fp32); the denominator comes from a
    ones column appended to V (out row 64); P^T V accumulates ctx^T
    [d, q] which feeds the output projection as lhsT directly.
  - Matmul inputs are float32r (full PE speed at N>=512); Q^T/K^T are
    bf16 with K^T zero-padded per head to a full 128-row contraction
    (K=64 matmuls stream at half rate on TRN2). 1/denom is computed as
    exp(-ln d) on ScalarE (same ACT table set as the softmax exp).
    Accumulation and softmax run in fp32.
"""

import numpy as np

import concourse.bass as bass  # noqa: F401
import concourse.mybir as mybir
import concourse.tile as tile
from concourse import bacc
from concourse.bass_interp import get_hw_module
from concourse.bass_utils import run_bass_kernel_spmd

B, L, H = 8, 1024, 768
NH, HD = 12, 64
NC = H // 128          # 6 chunks of hidden dim
LC = L // 128          # 8 chunks of sequence dim
F32 = mybir.dt.float32
EXP = mybir.ActivationFunctionType.Exp


def build_bass(compute_rounded: bool = True):
    CDT = mybir.dt.float32r if compute_rounded else F32

    nc = bacc.Bacc("TRN2", debug=False, num_devices=8)

    # The kernel uses Exp (softmax) and Ln (reciprocal via exp(-ln d)).
    # Walrus's table-load pass would alternate exp_and_others /
    # natural_log sets (a ~1.3us ACT table DMA per switch, dozens per
    # kernel). Make the combined natural_log_exp_and_others set the only
    # provider of Exp/Ln so a single table load serves the whole kernel.
    from concourse.hw_specs import get_activation_tables

    _tabs = get_activation_tables(nc.m.arch)
    _E = mybir.ActivationFunctionType.Exp
    _L = mybir.ActivationFunctionType.Ln
    if "natural_log_exp_and_others" in _tabs:
        for _name, _fns in _tabs.items():
            if _name != "natural_log_exp_and_others":
                _fns.discard(_E)
                _fns.discard(_L)

    xt_e = nc.declare_dram_parameter("xt", [H, L], CDT, isOutput=False)
    wqt_e = nc.declare_dram_parameter("wqt", [H, H], CDT, isOutput=False)
    wkt_e = nc.declare_dram_parameter("wkt", [H, H], CDT, isOutput=False)
    wvt_e = nc.declare_dram_parameter("wvt", [H, H], CDT, isOutput=False)
    wot_e = nc.declare_dram_parameter("wot", [H, H], CDT, isOutput=False)
    bq_e = nc.declare_dram_parameter("bq", [H], F32, isOutput=False)
    bk_e = nc.declare_dram_parameter("bk", [H], F32, isOutput=False)
    bv_e = nc.declare_dram_parameter("bv", [H], CDT, isOutput=False)
    bo_e = nc.declare_dram_parameter("bo", [H], CDT, isOutput=False)
    mask_e = nc.declare_dram_parameter("mask", [L], F32, isOutput=False)
    out_e = nc.declare_dram_parameter("out", [L, H], F32, isOutput=True)

    with tile.TileContext(nc) as tc:
        with (
            tc.tile_pool(name="small", bufs=1) as small,
            tc.tile_pool(name="acts", bufs=1) as acts,
            tc.tile_pool(name="outp", bufs=2) as out_pool,
            tc.tile_pool(name="psA", bufs=2, space="PSUM") as psA,
            tc.tile_pool(name="psB", bufs=2, space="PSUM") as psB,
        ):
            # ---- constants / small tensors ----
            mask_sb = small.tile([128, LC], F32)
            nc.sync.dma_start(mask_sb[:], mask_e[:].rearrange("(c p) -> p c", p=128))
            bq_sb = small.tile([128, NC], F32)
            nc.sync.dma_start(bq_sb[:], bq_e[:].rearrange("(c p) -> p c", p=128))
            bk_sb = small.tile([128, NC], F32)
            nc.sync.dma_start(bk_sb[:], bk_e[:].rearrange("(c p) -> p c", p=128))
            bv_sb = small.tile([1, H], CDT)
            nc.sync.dma_start(bv_sb[:], bv_e[None, :])
            bo_sb = small.tile([1, H], CDT)
            nc.sync.dma_start(bo_sb[:], bo_e[None, :])
            ones32 = small.tile([128, 128], F32)
            nc.vector.memset(ones32[:], 1.0)
            ones = small.tile([128, 128], CDT)
            nc.vector.tensor_copy(ones[:], ones32[:])

            BF = mybir.dt.bfloat16
            qt_sb = acts.tile([128, NC, L], BF)
            kt_sb = acts.tile([128, NH, L], BF)  # per-head K^T, other 64 rows zero
            nc.gpsimd.memset(kt_sb[:], 0.0)
            v_sb = acts.tile([128, LC, NH, HD + 1], CDT)  # [..., 64] = ones col
            ctxt_sb = acts.tile([128, NC, L], CDT)

            nc.vector.tensor_copy(
                v_sb[:, :, :, HD],
                ones32[:, 0 : LC * NH].rearrange("p (a b) -> p a b", a=LC),
            )

            # =========== projection phase (xt + wv/wk/wq scoped) ===========
            with (
                tc.tile_pool(name="xt", bufs=1) as xt_pool,
                tc.tile_pool(name="w1", bufs=2) as w1,
            ):
                # interleave xt/wv chunk DMAs: V-proj's first matmuls only
                # need chunk 0 of each, so compute starts ~3us in instead of
                # waiting for the full 5.25MB load
                xt_sb = xt_pool.tile([128, NC, L], CDT)
                wv_sb = w1.tile([128, NC, H], CDT, tag="w")
                for c in range(NC):
                    nc.sync.dma_start(
                        wv_sb[:, c, :],
                        wvt_e[:].rearrange("(c p) d -> p c d", p=128)[:, c, :],
                    )
                    nc.sync.dma_start(
                        xt_sb[:, c, :],
                        xt_e[:].rearrange("(c p) q -> p c q", p=128)[:, c, :],
                    )
                for lc in range(LC):
                    ps = psA.tile([128, 1024], F32, tag="psA")
                    for off, width in ((0, 512), (512, 256)):
                        for kc in range(NC):
                            nc.tensor.matmul(
                                ps[:, off : off + width],
                                xt_sb[:, kc, lc * 128 : lc * 128 + 128],
                                wv_sb[:, kc, off : off + width],
                                start=(kc == 0),
                                stop=False,
                            )
                        nc.tensor.matmul(  # + bv (ones row x bias row)
                            ps[:, off : off + width],
                            ones[0:1, 0:128],
                            bv_sb[0:1, off : off + width],
                            start=False,
                            stop=True,
                        )
                    nc.vector.tensor_copy(
                        v_sb[:, lc, :, 0:HD],
                        ps[:, 0:H].rearrange("p (h d) -> p h d", d=HD),
                    )

                # ---- K^T then Q^T projections: out[d, q] = W x^T + b
                for w_e, b_sb, dst in ((wkt_e, bk_sb, kt_sb), (wqt_e, bq_sb, qt_sb)):
                    w_sb = w1.tile([128, NC, H], CDT, tag="w")
                    for c in range(NC):
                        nc.sync.dma_start(
                            w_sb[:, c, :],
                            w_e[:].rearrange("(c p) d -> p c d", p=128)[:, c, :],
                        )
                    for dc in range(NC):
                        ps = psA.tile([128, 1024], F32, tag="psA")
                        for qh in range(2):
                            o = qh * 512
                            for kc in range(NC):
                                nc.tensor.matmul(
                                    ps[:, o : o + 512],
                                    w_sb[:, kc, dc * 128 : dc * 128 + 128],
                                    xt_sb[:, kc, o : o + 512],
                                    start=(kc == 0),
                                    stop=(kc == NC - 1),
                                )
                        if dst is qt_sb:
                            nc.vector.tensor_scalar_add(
                                dst[:, dc, :], ps[:, :], b_sb[:, dc : dc + 1]
                            )
                        else:
                            nc.vector.tensor_scalar_add(
                                kt_sb[0:64, 2 * dc, :], ps[0:64, :],
                                b_sb[0:64, dc : dc + 1],
                            )
                            nc.vector.tensor_scalar_add(
                                kt_sb[64:128, 2 * dc + 1, :], ps[64:128, :],
                                b_sb[64:128, dc : dc + 1],
                            )

            # =========== attention + output projection ===========
            with (
                tc.tile_pool(name="w2", bufs=1) as w2,
                tc.tile_pool(name="et", bufs=6) as et_pool,
                tc.tile_pool(name="norm", bufs=1) as norm_pool,
            ):
                wo_sb = w2.tile([128, NC, H], CDT)
                for c in range(NC):
                    nc.sync.dma_start(
                        wo_sb[:, c, :],
                        wot_e[:].rearrange("(c p) d -> p c d", p=128)[:, c, :],
                    )

                def make_normalize(hp, ctxu_a, ctxu_b, ra, rb):
                    def emit():
                        # broadcast 1/denom over 64 partitions via f32r matmul,
                        # then scale ctx^T and store to ctxt_sb
                        for recip, ctxu, btag in ((ra, ctxu_a, "bca"),
                                                  (rb, ctxu_b, "bcb")):
                            bc = psA.tile([64, 1024], F32, tag="psA")
                            for o in (0, 512):
                                nc.tensor.matmul(
                                    bc[:, o : o + 512],
                                    ones[64:65, 0:64],
                                    recip[64:65, o : o + 512],
                                    start=True,
                                    stop=True,
                                )
                            bc_sb = norm_pool.tile([64, 1024], F32, tag=btag)
                            nc.vector.tensor_copy(bc_sb[:], bc[:])
                            if btag == "bca":
                                nc.gpsimd.tensor_tensor(
                                    ctxt_sb[0:64, hp, :], ctxu[0:64, :], bc_sb[:],
                                    mybir.AluOpType.mult,
                                )
                            else:
                                tmp_o = norm_pool.tile([64, 1024], CDT, tag="tmp")
                                nc.gpsimd.tensor_tensor(
                                    tmp_o[:], ctxu[0:64, :], bc_sb[:],
                                    mybir.AluOpType.mult,
                                )
                                # lift odd head to partitions 64:128 (DMA can
                                # cross partitions; DVE cannot)
                                nc.sync.dma_start(ctxt_sb[64:128, hp, :], tmp_o[:])
                    return emit

                pending = None
                for hp in range(NH // 2):
                    ha, hb = 2 * hp, 2 * hp + 1
                    # per head: rows 0:64 = ctx^T, row 64 = softmax denominator
                    ctx_a = psB.tile([128, 1024], F32, tag="psB")
                    ctx_b = psB.tile([128, 1024], F32, tag="psB")

                    def emit_pv(kc, et_a, et_b, ctx_a=ctx_a, ctx_b=ctx_b,
                                ha=ha, hb=hb):
                        first, last = kc == 0, kc == LC - 1
                        for qh in range(2):
                            o = qh * 512
                            # ctx^T[d, q] += V^T P^T ; row 64 = denominator
                            nc.tensor.matmul(
                                ctx_a[0 : HD + 1, o : o + 512],
                                v_sb[:, kc, ha, :],
                                et_a[:, o : o + 512],
                                start=first,
                                stop=last,
                            )
                            nc.tensor.matmul(
                                ctx_b[0 : HD + 1, o : o + 512],
                                v_sb[:, kc, hb, :],
                                et_b[:, o : o + 512],
                                start=first,
                                stop=last,
                            )

                    pv_q = []  # software pipeline: PV(kc-1) after ST(kc)
                    for kc in range(LC):
                        st_a = psA.tile([128, 1024], F32, tag="psA")
                        st_b = psA.tile([128, 1024], F32, tag="psA")
                        for qh in range(2):
                            o = qh * 512
                            # S^T[k, q] = K Q^T for both heads (row-group packed)
                            nc.tensor.matmul(
                                st_a[:, o : o + 512],
                                kt_sb[:, ha, kc * 128 : kc * 128 + 128],
                                qt_sb[:, hp, o : o + 512],
                                start=True,
                                stop=True,
                            )
                            nc.tensor.matmul(
                                st_b[:, o : o + 512],
                                kt_sb[:, hb, kc * 128 : kc * 128 + 128],
                                qt_sb[:, hp, o : o + 512],
                                start=True,
                                stop=True,
                            )
                        # P^T = exp(S^T/8 + mask_k)
                        et_a = et_pool.tile([128, 1024], CDT, tag="et")
                        et_b = et_pool.tile([128, 1024], CDT, tag="et")
                        nc.scalar.activation(
                            et_a[:], st_a[:], EXP,
                            bias=mask_sb[:, kc : kc + 1], scale=0.125,
                        )
                        nc.scalar.activation(
                            et_b[:], st_b[:], EXP,
                            bias=mask_sb[:, kc : kc + 1], scale=0.125,
                        )
                        pv_q.append((kc, et_a, et_b))
                        if kc >= 1:
                            emit_pv(*pv_q.pop(0))
                        if kc == 3 and pending is not None:
                            # previous pair's normalize, emitted here so its
                            # matmuls never head-of-line-block the PE
                            pending()
                            pending = None
                    emit_pv(*pv_q.pop(0))
                    # evacuate ctx+denominator to SBUF immediately: frees the
                    # PSUM slot and takes the reciprocal off the PE path
                    ctxu_a = norm_pool.tile([65, 1024], F32, tag="cua")
                    nc.vector.tensor_copy(ctxu_a[:], ctx_a[0:65, :])
                    ctxu_b = norm_pool.tile([65, 1024], F32, tag="cub")
                    nc.vector.tensor_copy(ctxu_b[:], ctx_b[0:65, :])
                    # 1/d = exp(-ln d) on ScalarE: Log+Exp share one ACT
                    # table set, ~1e-5 rel err, and it keeps the reciprocal
                    # off the (slow, 1-lane) DVE path entirely
                    LOG = mybir.ActivationFunctionType.Ln
                    ra = norm_pool.tile([65, 1024], CDT, tag="ra")
                    if hp == NH // 2 - 1:
                        # last pair: no following matmul stream hides the DVE
                        # reciprocal's ~8us latency; use the short ACT path
                        lna = norm_pool.tile([65, 1024], F32, tag="lna")
                        nc.scalar.activation(lna[64:65, :], ctxu_a[64:65, :], LOG)
                        nc.scalar.activation(
                            ra[64:65, :], lna[64:65, :], EXP, scale=-1.0
                        )
                    else:
                        ra32 = norm_pool.tile([65, 1024], F32, tag="ra32")
                        nc.vector.reciprocal(ra32[64:65, :], ctxu_a[64:65, :])
                        nc.vector.tensor_copy(ra[64:65, :], ra32[64:65, :])
                    lnb = norm_pool.tile([65, 1024], F32, tag="lnb")
                    nc.scalar.activation(lnb[64:65, :], ctxu_b[64:65, :], LOG)
                    rb = norm_pool.tile([65, 1024], CDT, tag="rb")
                    nc.scalar.activation(rb[64:65, :], lnb[64:65, :], EXP, scale=-1.0)
                    pending = make_normalize(hp, ctxu_a, ctxu_b, ra, rb)
                if pending is not None:
                    pending()
                    pending = None

                # ---- output projection: out[q, o] = ctx Wo^T + bo ----
                for lc in range(LC):
                    ps = psA.tile([128, 1024], F32, tag="psA")
                    for off, width in ((0, 512), (512, 256)):
                        for c in range(NC):
                            nc.tensor.matmul(
                                ps[:, off : off + width],
                                ctxt_sb[:, c, lc * 128 : lc * 128 + 128],
                                wo_sb[:, c, off : off + width],
                                start=(c == 0),
                                stop=False,
                            )
                        nc.tensor.matmul(  # + bo
                            ps[:, off : off + width],
                            ones[0:1, 0:128],
                            bo_sb[0:1, off : off + width],
                            start=False,
                            stop=True,
                        )
                    o_sb = out_pool.tile([128, H], F32, tag="outp")
                    nc.vector.tensor_copy(o_sb[:], ps[:, 0:H])
                    nc.sync.dma_start(out_e[lc * 128 : lc * 128 + 128, :], o_sb[:])

    nc.finalize()
    nc.m = get_hw_module(nc.m)
    return nc


_NC_CACHE = {}


def _get_nc(compute_rounded: bool = True):
    if compute_rounded not in _NC_CACHE:
        _NC_CACHE[compute_rounded] = build_bass(compute_rounded)
    return _NC_CACHE[compute_rounded]


def make_in_maps(inputs):
    f = lambda a: np.ascontiguousarray(np.asarray(a, dtype=np.float32))  # noqa: E731
    hs = f(inputs["hidden_states"])
    mask = f(inputs["attention_mask"]).reshape(B, L)
    shared = {
        "wqt": f(np.asarray(inputs["Wq"]).T),
        "wkt": f(np.asarray(inputs["Wk"]).T),
        "wvt": f(np.asarray(inputs["Wv"]).T),
        "wot": f(np.asarray(inputs["Wo"]).T),
        "bq": f(inputs["bq"]),
        "bk": f(inputs["bk"]),
        "bv": f(inputs["bv"]),
        "bo": f(inputs["bo"]),
    }
    return [
        {"xt": f(hs[b].T), "mask": mask[b], **shared}
        for b in range(B)
    ]


def run_spmd(inputs, trace=False, compute_rounded=True):
    nc = _get_nc(compute_rounded)
    res = run_bass_kernel_spmd(nc, make_in_maps(inputs), list(range(B)), trace=trace)
    out = np.stack([res.results[b]["out"] for b in range(B)]).astype(np.float32)
    return out, res


def kernel(**inputs) -> np.ndarray:
    out, _ = run_spmd(inputs, trace=False)
    return out


# revision 17
# speedup vs baseline: 1.5313x; 1.0139x over previous
"""BERT self-attention on 8 Trainium2 NeuronCores.

Sharding: data-parallel over batch (B=8 -> one batch element per core).
Each core computes full self-attention for its batch element:
  Q/K/V projections, per-head softmax(Q K^T / 8 + mask) V, output proj.

Layout strategy (per core):
  - Host passes xT = x.T [768,1024] and W.T [768,768] so every matmul
    contracts over the partition axis.
  - QT,KT [d, L] and V [L, d] are produced directly by the projections.
  - Attention runs transposed: ST[k,q] = K Q^T per head, so softmax's
    reduction axis (k) lands on partitions: exp via ScalarE with the
    attention mask as per-partition bias (no max subtraction: scores are
    ~N(0,1), |s|<~7, exp is safe in fp32); the denominator comes from a
    ones column appended to V (out row 64); P^T V accumulates ctx^T
    [d, q] which feeds the output projection as lhsT directly.
  - Matmul inputs are float32r (full PE speed at N>=512); Q^T/K^T are
    bf16 with K^T zero-padded per head to a full 128-row contraction
    (K=64 matmuls stream at half rate on TRN2). 1/denom is computed as
    exp(-ln d) on ScalarE (same ACT table set as the softmax exp).
    Accumulation and softmax run in fp32.
"""

import numpy as np

import concourse.bass as bass  # noqa: F401
import concourse.mybir as mybir
import concourse.tile as tile
from concourse import bacc
from concourse.bass_interp import get_hw_module
from concourse.bass_utils import run_bass_kernel_spmd

B, L, H = 8, 1024, 768
NH, HD = 12, 64
NC = H // 128          # 6 chunks of hidden dim
LC = L // 128          # 8 chunks of sequence dim
F32 = mybir.dt.float32
EXP = mybir.ActivationFunctionType.Exp


def build_bass(compute_rounded: bool = True):
    CDT = mybir.dt.float32r if compute_rounded else F32

    nc = bacc.Bacc("TRN2", debug=False, num_devices=8)

    # The kernel uses Exp (softmax) and Ln (reciprocal via exp(-ln d)).
    # Walrus's table-load pass would alternate exp_and_others /
    # natural_log sets (a ~1.3us ACT table DMA per switch, dozens per
    # kernel). Make the combined natural_log_exp_and_others set the only
    # provider of Exp/Ln so a single table load serves the whole kernel.
    from concourse.hw_specs import get_activation_tables

    _tabs = get_activation_tables(nc.m.arch)
    _E = mybir.ActivationFunctionType.Exp
    _L = mybir.ActivationFunctionType.Ln
    if "natural_log_exp_and_others" in _tabs:
        for _name, _fns in _tabs.items():
            if _name != "natural_log_exp_and_others":
                _fns.discard(_E)
                _fns.discard(_L)

    xt_e = nc.declare_dram_parameter("xt", [H, L], CDT, isOutput=False)
    wqt_e = nc.declare_dram_parameter("wqt", [H, H], CDT, isOutput=False)
    wkt_e = nc.declare_dram_parameter("wkt", [H, H], CDT, isOutput=False)
    wvt_e = nc.declare_dram_parameter("wvt", [H, H], CDT, isOutput=False)
    wot_e = nc.declare_dram_parameter("wot", [H, H], CDT, isOutput=False)
    bq_e = nc.declare_dram_parameter("bq", [H], F32, isOutput=False)
    bk_e = nc.declare_dram_parameter("bk", [H], F32, isOutput=False)
    bv_e = nc.declare_dram_parameter("bv", [H], CDT, isOutput=False)
    bo_e = nc.declare_dram_parameter("bo", [H], CDT, isOutput=False)
    mask_e = nc.declare_dram_parameter("mask", [L], F32, isOutput=False)
    out_e = nc.declare_dram_parameter("out", [L, H], F32, isOutput=True)

    with tile.TileContext(nc) as tc:
        with (
            tc.tile_pool(name="small", bufs=1) as small,
            tc.tile_pool(name="acts", bufs=1) as acts,
            tc.tile_pool(name="outp", bufs=2) as out_pool,
            tc.tile_pool(name="psA", bufs=2, space="PSUM") as psA,
            tc.tile_pool(name="psB", bufs=2, space="PSUM") as psB,
        ):
            # ---- constants / small tensors ----
            mask_sb = small.tile([128, LC], F32)
            nc.sync.dma_start(mask_sb[:], mask_e[:].rearrange("(c p) -> p c", p=128))
            bq_sb = small.tile([128, NC], F32)
            nc.sync.dma_start(bq_sb[:], bq_e[:].rearrange("(c p) -> p c", p=128))
            bk_sb = small.tile([128, NC], F32)
            nc.sync.dma_start(bk_sb[:], bk_e[:].rearrange("(c p) -> p c", p=128))
            bv_sb = small.tile([1, H], CDT)
            nc.sync.dma_start(bv_sb[:], bv_e[None, :])
            bo_sb = small.tile([1, H], CDT)
            nc.sync.dma_start(bo_sb[:], bo_e[None, :])
            ones32 = small.tile([128, 128], F32)
            nc.vector.memset(ones32[:], 1.0)
            ones = small.tile([128, 128], CDT)
            nc.vector.tensor_copy(ones[:], ones32[:])

            BF = mybir.dt.bfloat16
            qt_sb = acts.tile([128, NC, L], BF)
            kt_sb = acts.tile([128, NH, L], BF)  # per-head K^T, other 64 rows zero
            nc.gpsimd.memset(kt_sb[:], 0.0)
            v_sb = acts.tile([128, LC, NH, HD + 1], CDT)  # [..., 64] = ones col
            ctxt_sb = acts.tile([128, NC, L], CDT)

            nc.vector.tensor_copy(
                v_sb[:, :, :, HD],
                ones32[:, 0 : LC * NH].rearrange("p (a b) -> p a b", a=LC),
            )

            # =========== projection phase (xt + wv/wk/wq scoped) ===========
            with (
                tc.tile_pool(name="xt", bufs=1) as xt_pool,
                tc.tile_pool(name="w1", bufs=2) as w1,
            ):
                # interleave xt/wv chunk DMAs: V-proj's first matmuls only
                # need chunk 0 of each, so compute starts ~3us in instead of
                # waiting for the full 5.25MB load
                xt_sb = xt_pool.tile([128, NC, L], CDT)
                wv_sb = w1.tile([128, NC, H], CDT, tag="w")
                for c in range(NC):
                    nc.sync.dma_start(
                        wv_sb[:, c, :],
                        wvt_e[:].rearrange("(c p) d -> p c d", p=128)[:, c, :],
                    )
                    nc.sync.dma_start(
                        xt_sb[:, c, :],
                        xt_e[:].rearrange("(c p) q -> p c q", p=128)[:, c, :],
                    )
                for lc in range(LC):
                    ps = psA.tile([128, 1024], F32, tag="psA")
                    for off, width in ((0, 512), (512, 256)):
                        for kc in range(NC):
                            nc.tensor.matmul(
                                ps[:, off : off + width],
                                xt_sb[:, kc, lc * 128 : lc * 128 + 128],
                                wv_sb[:, kc, off : off + width],
                                start=(kc == 0),
                                stop=False,
                            )
                        nc.tensor.matmul(  # + bv (ones row x bias row)
                            ps[:, off : off + width],
                            ones[0:1, 0:128],
                            bv_sb[0:1, off : off + width],
                            start=False,
                            stop=True,
                        )
                    nc.vector.tensor_copy(
                        v_sb[:, lc, :, 0:HD],
                        ps[:, 0:H].rearrange("p (h d) -> p h d", d=HD),
                    )

                # ---- K^T then Q^T projections: out[d, q] = W x^T + b
                for w_e, b_sb, dst in ((wkt_e, bk_sb, kt_sb), (wqt_e, bq_sb, qt_sb)):
                    w_sb = w1.tile([128, NC, H], CDT, tag="w")
                    for c in range(NC):
                        nc.sync.dma_start(
                            w_sb[:, c, :],
                            w_e[:].rearrange("(c p) d -> p c d", p=128)[:, c, :],
                        )
                    for dc in range(NC):
                        ps = psA.tile([128, 1024], F32, tag="psA")
                        for qh in range(2):
                            o = qh * 512
                            for kc in range(NC):
                                nc.tensor.matmul(
                                    ps[:, o : o + 512],
                                    w_sb[:, kc, dc * 128 : dc * 128 + 128],
                                    xt_sb[:, kc, o : o + 512],
                                    start=(kc == 0),
                                    stop=(kc == NC - 1),
                                )
                        if dst is qt_sb:
                            nc.vector.tensor_scalar_add(
                                dst[:, dc, :], ps[:, :], b_sb[:, dc : dc + 1]
                            )
                        else:
                            nc.vector.tensor_scalar_add(
                                kt_sb[0:64, 2 * dc, :], ps[0:64, :],
                                b_sb[0:64, dc : dc + 1],
                            )
                            nc.vector.tensor_scalar_add(
                                kt_sb[64:128, 2 * dc + 1, :], ps[64:128, :],
                                b_sb[64:128, dc : dc + 1],
                            )

            # =========== attention + output projection ===========
            with (
                tc.tile_pool(name="w2", bufs=1) as w2,
                tc.tile_pool(name="et", bufs=6) as et_pool,
                tc.tile_pool(name="norm", bufs=1) as norm_pool,
            ):
                wo_sb = w2.tile([128, NC, H], CDT)
                for c in range(NC):
                    nc.sync.dma_start(
                        wo_sb[:, c, :],
                        wot_e[:].rearrange("(c p) d -> p c d", p=128)[:, c, :],
                    )

                def make_normalize(hp, ctxu_a, ctxu_b, ra, rb):
                    def emit():
                        # broadcast 1/denom over 64 partitions via f32r matmul,
                        # then scale ctx^T and store to ctxt_sb
                        for recip, ctxu, btag in ((ra, ctxu_a, "bca"),
                                                  (rb, ctxu_b, "bcb")):
                            bc = psA.tile([64, 1024], F32, tag="psA")
                            for o in (0, 512):
                                nc.tensor.matmul(
                                    bc[:, o : o + 512],
                                    ones[64:65, 0:64],
                                    recip[64:65, o : o + 512],
                                    start=True,
                                    stop=True,
                                )
                            bc_sb = norm_pool.tile([64, 1024], F32, tag=btag)
                            nc.vector.tensor_copy(bc_sb[:], bc[:])
                            if btag == "bca":
                                nc.gpsimd.tensor_tensor(
                                    ctxt_sb[0:64, hp, :], ctxu[0:64, :], bc_sb[:],
                                    mybir.AluOpType.mult,
                                )
                            else:
                                tmp_o = norm_pool.tile([64, 1024], CDT, tag="tmp")
                                nc.gpsimd.tensor_tensor(
                                    tmp_o[:], ctxu[0:64, :], bc_sb[:],
                                    mybir.AluOpType.mult,
                                )
                                # lift odd head to partitions 64:128 (DMA can
                                # cross partitions; DVE cannot)
                                nc.sync.dma_start(ctxt_sb[64:128, hp, :], tmp_o[:])
                    return emit

                pending = None
                pend_recips = None
                for hp in range(NH // 2):
                    ha, hb = 2 * hp, 2 * hp + 1
                    # per head: rows 0:64 = ctx^T, row 64 = softmax denominator
                    ctx_a = psB.tile([128, 1024], F32, tag="psB")
                    ctx_b = psB.tile([128, 1024], F32, tag="psB")

                    def emit_pv(kc, et_a, et_b, ctx_a=ctx_a, ctx_b=ctx_b,
                                ha=ha, hb=hb):
                        first, last = kc == 0, kc == LC - 1
                        for qh in range(2):
                            o = qh * 512
                            # ctx^T[d, q] += V^T P^T ; row 64 = denominator
                            nc.tensor.matmul(
                                ctx_a[0 : HD + 1, o : o + 512],
                                v_sb[:, kc, ha, :],
                                et_a[:, o : o + 512],
                                start=first,
                                stop=last,
                            )
                            nc.tensor.matmul(
                                ctx_b[0 : HD + 1, o : o + 512],
                                v_sb[:, kc, hb, :],
                                et_b[:, o : o + 512],
                                start=first,
                                stop=last,
                            )

                    pv_q = []  # software pipeline: PV(kc-1) after ST(kc)
                    for kc in range(LC):
                        st_a = psA.tile([128, 1024], F32, tag="psA")
                        st_b = psA.tile([128, 1024], F32, tag="psA")
                        for qh in range(2):
                            o = qh * 512
                            # S^T[k, q] = K Q^T for both heads (row-group packed)
                            nc.tensor.matmul(
                                st_a[:, o : o + 512],
                                kt_sb[:, ha, kc * 128 : kc * 128 + 128],
                                qt_sb[:, hp, o : o + 512],
                                start=True,
                                stop=True,
                            )
                            nc.tensor.matmul(
                                st_b[:, o : o + 512],
                                kt_sb[:, hb, kc * 128 : kc * 128 + 128],
                                qt_sb[:, hp, o : o + 512],
                                start=True,
                                stop=True,
                            )
                        # P^T = exp(S^T/8 + mask_k)
                        et_a = et_pool.tile([128, 1024], CDT, tag="et")
                        et_b = et_pool.tile([128, 1024], CDT, tag="et")
                        nc.scalar.activation(
                            et_a[:], st_a[:], EXP,
                            bias=mask_sb[:, kc : kc + 1], scale=0.125,
                        )
                        nc.scalar.activation(
                            et_b[:], st_b[:], EXP,
                            bias=mask_sb[:, kc : kc + 1], scale=0.125,
                        )
                        pv_q.append((kc, et_a, et_b))
                        if kc >= 1:
                            emit_pv(*pv_q.pop(0))
                        if kc == 1 and pend_recips is not None:
                            # previous pair's head-b ACT recip: emitted after
                            # this pair's first exps so it doesn't delay them
                            # in the ScalarE FIFO (st-slot recycling couples
                            # those exps to the PE's ST stream)
                            pend_recips()
                            pend_recips = None
                        if kc == 5 and pending is not None:
                            # previous pair's normalize, emitted here so its
                            # matmuls never head-of-line-block the PE
                            pending()
                            pending = None
                    emit_pv(*pv_q.pop(0))
                    # evacuate ctx+denominator to SBUF immediately: frees the
                    # PSUM slot and takes the reciprocal off the PE path
                    ctxu_a = norm_pool.tile([65, 1024], F32, tag="cua")
                    nc.vector.tensor_copy(ctxu_a[:], ctx_a[0:65, :])
                    ctxu_b = norm_pool.tile([65, 1024], F32, tag="cub")
                    nc.vector.tensor_copy(ctxu_b[:], ctx_b[0:65, :])
                    # 1/d = exp(-ln d) on ScalarE: Log+Exp share one ACT
                    # table set, ~1e-5 rel err, and it keeps the reciprocal
                    # off the (slow, 1-lane) DVE path entirely
                    LOG = mybir.ActivationFunctionType.Ln
                    ra = norm_pool.tile([65, 1024], CDT, tag="ra")
                    rb = norm_pool.tile([65, 1024], CDT, tag="rb")
                    if hp == NH // 2 - 1:
                        # last pair: no following matmul stream hides the DVE
                        # reciprocal's ~8us latency; use the short ACT path
                        lna = norm_pool.tile([65, 1024], F32, tag="lna")
                        nc.scalar.activation(lna[64:65, :], ctxu_a[64:65, :], LOG)
                        nc.scalar.activation(
                            ra[64:65, :], lna[64:65, :], EXP, scale=-1.0
                        )
                    else:
                        ra32 = norm_pool.tile([65, 1024], F32, tag="ra32")
                        nc.vector.reciprocal(ra32[64:65, :], ctxu_a[64:65, :])
                        nc.vector.tensor_copy(ra[64:65, :], ra32[64:65, :])

                    def emit_recip_b(ctxu_b=ctxu_b, rb=rb):
                        lnb = norm_pool.tile([65, 1024], F32, tag="lnb")
                        nc.scalar.activation(lnb[64:65, :], ctxu_b[64:65, :], LOG)
                        nc.scalar.activation(
                            rb[64:65, :], lnb[64:65, :], EXP, scale=-1.0
                        )

                    if hp == NH // 2 - 1:
                        emit_recip_b()
                    else:
                        pend_recips = emit_recip_b
                    pending = make_normalize(hp, ctxu_a, ctxu_b, ra, rb)
                if pend_recips is not None:
                    pend_recips()
                    pend_recips = None
                if pending is not None:
                    pending()
                    pending = None

                # ---- output projection: out[q, o] = ctx Wo^T + bo ----
                for lc in range(LC):
                    ps = psA.tile([128, 1024], F32, tag="psA")
                    for off, width in ((0, 512), (512, 256)):
                        for c in range(NC):
                            nc.tensor.matmul(
                                ps[:, off : off + width],
                                ctxt_sb[:, c, lc * 128 : lc * 128 + 128],
                                wo_sb[:, c, off : off + width],
                                start=(c == 0),
                                stop=False,
                            )
                        nc.tensor.matmul(  # + bo
                            ps[:, off : off + width],
                            ones[0:1, 0:128],
                            bo_sb[0:1, off : off + width],
                            start=False,
                            stop=True,
                        )
                    o_sb = out_pool.tile([128, H], F32, tag="outp")
                    nc.vector.tensor_copy(o_sb[:], ps[:, 0:H])
                    nc.sync.dma_start(out_e[lc * 128 : lc * 128 + 128, :], o_sb[:])

    nc.finalize()
    nc.m = get_hw_module(nc.m)
    return nc


_NC_CACHE = {}


def _get_nc(compute_rounded: bool = True):
    if compute_rounded not in _NC_CACHE:
        _NC_CACHE[compute_rounded] = build_bass(compute_rounded)
    return _NC_CACHE[compute_rounded]


def make_in_maps(inputs):
    f = lambda a: np.ascontiguousarray(np.asarray(a, dtype=np.float32))  # noqa: E731
    hs = f(inputs["hidden_states"])
    mask = f(inputs["attention_mask"]).reshape(B, L)
    shared = {
        "wqt": f(np.asarray(inputs["Wq"]).T),
        "wkt": f(np.asarray(inputs["Wk"]).T),
        "wvt": f(np.asarray(inputs["Wv"]).T),
        "wot": f(np.asarray(inputs["Wo"]).T),
        "bq": f(inputs["bq"]),
        "bk": f(inputs["bk"]),
        "bv": f(inputs["bv"]),
        "bo": f(inputs["bo"]),
    }
    return [
        {"xt": f(hs[b].T), "mask": mask[b], **shared}
        for b in range(B)
    ]


def run_spmd(inputs, trace=False, compute_rounded=True):
    nc = _get_nc(compute_rounded)
    res = run_bass_kernel_spmd(nc, make_in_maps(inputs), list(range(B)), trace=trace)
    out = np.stack([res.results[b]["out"] for b in range(B)]).astype(np.float32)
    return out, res


def kernel(**inputs) -> np.ndarray:
    out, _ = run_spmd(inputs, trace=False)
    return out
